# revision 3
# baseline (speedup 1.0000x reference)
"""CapsAlexNet (FLOWER102) forward pass on 8 Trainium2 NeuronCores — v2.

Sharding (same global structure as v1, heavily bf16 + restructured):
  - conv stack: data-parallel over batch (2 images/core); conv1 via host
    im2col; all matmuls bf16 (fp32 PSUM accumulate).
  - capsule routing: capsule dim sharded 8 ways (AllToAll from batch-shard
    to i-shard). x_hat (X) computed ONCE in bf16 and kept resident in SBUF
    (17.5MB); the two logit/softmax passes run whole-X DVE ops in chunks of
    CH groups. AllReduce of [16,1632] bf16 per routing iteration (3 total).
  - caps conv computed fully on every core (v is global after AllReduce).
  - FC head: FC1 output-sharded (512 cols/core, bf16 weights streamed),
    FC2 input-sharded with a single AllReduce of the pre-activation,
    FC3 computed fully on every core.
  Collectives: AllToAll + 3x AllReduce + 1x AllReduce = 5.
"""

import numpy as np
from numpy.lib.stride_tricks import as_strided

import concourse.bass as bass
import concourse.mybir as mybir
import concourse.tile as tile
from concourse import bacc
from concourse.ap import AP
from concourse.bass_utils import run_bass_kernel_spmd

F32 = mybir.dt.float32
BF16 = mybir.dt.bfloat16
AX = mybir.AxisListType
OP = mybir.AluOpType
AF = mybir.ActivationFunctionType

NCORES = 8
B = 16
BC = 2           # images per core
O = 102
D = 16
OD = O * D       # 1632
ITOT = 2592
IPAD = 2688
ILOC = IPAD // NCORES   # 336
G = ILOC // 8           # 42 groups of 8 caps
CH = 4                  # groups per DVE chunk in routing passes
RG = [list(range(NCORES))]

_CACHE = {}


def _dap(a, offset, dims):
    """Manual AP into a dram-pool tile (which is itself an AP)."""
    return AP(tensor=a.tensor, offset=a.offset + offset,
              ap=[list(d) for d in dims])


def _pv(t, part0, free0, dims):
    """AP into SBUF tile t at (partition part0, free offset free0).

    dims: list of [step, count] free dims; prepend ("P", n) to set the
    partition count (default: full tile partitions).
    """
    base = t[:]
    fs = base.ap[0][0]          # partition stride == free size
    npart = dims[0][1] if dims[0][0] == "P" else base.ap[0][1]
    rest = dims[1:] if dims[0][0] == "P" else dims
    return AP(tensor=base.tensor, offset=base.offset + part0 * fs + free0,
              ap=[[fs, npart]] + [list(d) for d in rest])


def build_program():
    nc = bacc.Bacc("TRN2", target_bir_lowering=False, debug=False,
                   num_devices=NCORES)

    def din(name, shape, dt=F32):
        return nc.declare_dram_parameter(name, list(shape), dt, isOutput=False)

    T = dict(
        xcols=din("xcols", [BC, 363, 2601], BF16),
        w1T=din("w1T", [363, 96], BF16), b1c=din("b1c", [96, 1]),
        w2T=din("w2T", [25, 96, 256], BF16), b2c=din("b2c", [2, 128, 1]),
        wpT=din("wpT", [16, 2, 128, 256], BF16), bpc=din("bpc", [2, 128, 1]),
        wrg=din("wrg", [G, 64, OD], BF16),
        smat=din("smat", [128, 16], BF16),
        w3T=din("w3T", [3, 768], BF16), b3c=din("b3c", [2, 128, 1]),
        fw1T=din("fw1T", [18, 128, 8 * 512], BF16),
        fb1r=din("fb1r", [16, 512]),
        fw2T=din("fw2T", [4, 128, 4096], BF16),
        fb2T=din("fb2T", [128, 32]),
        fw3T=din("fw3T", [32, 128, 102], BF16),
        fb3r=din("fb3r", [16, 102]),
    )
    T["out_t"] = nc.declare_dram_parameter("out", [B, O], F32, isOutput=True)

    with tile.TileContext(nc) as tc:
        with tc.tile_pool(name="dram", bufs=1, space="DRAM") as dram:
            _build_body(nc, tc, dram, T)
    nc.finalize()
    return nc


def _build_body(nc, tc, dram, T):
    out_t = T["out_t"]

    # ---------------- DRAM scratch ----------------
    upc = dram.tile([BC, 20736], F32, tag="upc")
    u_loc = dram.tile([BC, IPAD * 8], BF16, tag="uloc")
    u_a2a = dram.tile([NCORES, BC, ILOC * 8], BF16, tag="ua2a")
    u_mine = dram.tile([NCORES, BC, ILOC * 8], BF16, tag="umine")
    u_mT = dram.tile([ILOC * 8, B], BF16, tag="umT")
    v_in = [dram.tile([16, OD], BF16, tag=f"vin{i}", name=f"vin{i}")
            for i in range(3)]
    v_out = [dram.tile([16, OD], BF16, tag=f"vout{i}", name=f"vout{i}")
             for i in range(3)]
    v2d = dram.tile([B * OD], BF16, tag="v2d")
    f1T = dram.tile([512, B], BF16, tag="f1T")
    z2in = dram.tile([16, 4096], BF16, tag="z2in")
    z2out = dram.tile([16, 4096], BF16, tag="z2out")

    # =========================================================
    # Phase A: conv stack (2 images, bf16)
    # =========================================================
    with (
        tc.tile_pool(name="caw", bufs=1) as cw,
        tc.tile_pool(name="cact", bufs=1) as ca,
        tc.tile_pool(name="cps", bufs=2, space="PSUM") as cps,
        tc.tile_pool(name="cps1", bufs=1, space="PSUM") as cps1,
    ):
        # conv1 inputs + weights first (everything else overlaps conv1)
        xc_sb = ca.tile([128, BC * 3 * 2601], BF16, tag="xc")
        for img in range(BC):
            for kt in range(3):
                rows = 128 if kt < 2 else 107
                c0 = (img * 3 + kt) * 2601
                nc.sync.dma_start(out=xc_sb[:rows, c0:c0 + 2601],
                                  in_=T["xcols"][img, kt * 128:kt * 128 + rows, :])
        w1t_sb = cw.tile([128, 3 * 96], BF16, tag="w1t")
        for kt in range(3):
            rows = 128 if kt < 2 else 107
            nc.sync.dma_start(out=w1t_sb[:rows, kt * 96:(kt + 1) * 96],
                              in_=T["w1T"][kt * 128:kt * 128 + rows, :])
        b1_sb = cw.tile([96, 1], F32, tag="b1s")
        nc.sync.dma_start(out=b1_sb[:], in_=T["b1c"][:, :])
        w2t_sb = cw.tile([96, 25 * 256], BF16, tag="w2t")
        nc.sync.dma_start(out=w2t_sb[:].rearrange("p (t o) -> p t o", o=256),
                          in_=T["w2T"].ap().rearrange("t c o -> c t o"))
        wpt_sb = cw.tile([128, 32 * 256], BF16, tag="wpt")
        nc.sync.dma_start(
            out=wpt_sb[:].rearrange("p (t k o) -> p t k o", k=2, o=256),
            in_=T["wpT"].ap().rearrange("t k c o -> c t k o"))
        b2_sb = cw.tile([128, 2], F32, tag="b2s")
        nc.sync.dma_start(out=b2_sb[:].rearrange("c (m one) -> c m one", one=1),
                          in_=T["b2c"].ap().rearrange("m c one -> c m one"))
        bp_sb = cw.tile([128, 2], F32, tag="bps")
        nc.sync.dma_start(out=bp_sb[:].rearrange("c (m one) -> c m one", one=1),
                          in_=T["bpc"].ap().rearrange("m c one -> c m one"))

        # ---- conv1 + relu ----
        h1_sb = ca.tile([96, BC * 2601], BF16, tag="h1")
        for img in range(BC):
            for (n0, n1) in ((0, 512), (512, 1024), (1024, 1536),
                             (1536, 2048), (2048, 2560), (2560, 2601)):
                ps = cps.tile([96, 512], F32, tag="ps1")
                for kt in range(3):
                    rows = 128 if kt < 2 else 107
                    c0 = (img * 3 + kt) * 2601
                    nc.tensor.matmul(ps[:, :n1 - n0],
                                     w1t_sb[:rows, kt * 96:(kt + 1) * 96],
                                     xc_sb[:rows, c0 + n0:c0 + n1],
                                     start=(kt == 0), stop=(kt == 2))
                nc.scalar.activation(h1_sb[:, img * 2601 + n0:img * 2601 + n1],
                                     ps[:, :n1 - n0], AF.Relu, bias=b1_sb[:, 0:1])

        # ---- maxpool1 -> padded conv2 input ----
        p1p_sb = ca.tile([96, BC * 841], BF16, tag="p1p")
        nc.vector.memset(p1p_sb[:], 0.0)
        for img in range(BC):
            eng = nc.vector
            def h1v(ky, kx):
                return _pv(h1_sb, 0, img * 2601 + ky * 51 + kx,
                           [[102, 25], [2, 25]])
            dst = _pv(p1p_sb, 0, img * 841 + 2 * 29 + 2, [[29, 25], [1, 25]])
            eng.tensor_max(dst, h1v(0, 0), h1v(0, 1))
            for t in range(2, 9):
                ky, kx = divmod(t, 3)
                eng.tensor_max(dst, dst, h1v(ky, kx))

        # ---- conv2 + relu ----
        h2_sb = ca.tile([128, 2 * BC * 625], BF16, tag="h2")
        for mch in range(2):
            ps2 = {}
            for img in range(BC):
                for nch in range(2):
                    ps2[(img, nch)] = cps1.tile(
                        [128, 512], F32, tag=f"ps2_{img}_{nch}",
                        name=f"ps2_{mch}_{img}_{nch}")
            for tap in range(25):
                ky, kx = divmod(tap, 5)
                lhs = w2t_sb[:, tap * 256 + mch * 128:tap * 256 + (mch + 1) * 128]
                for img in range(BC):
                    for nch, (oy0, nyy) in enumerate(((0, 13), (13, 12))):
                        rhs = _pv(p1p_sb, 0,
                                  img * 841 + (oy0 + ky) * 29 + kx,
                                  [[29, nyy], [1, 25]])
                        nc.tensor.matmul(ps2[(img, nch)][:, :nyy * 25], lhs,
                                         rhs, start=(tap == 0), stop=(tap == 24))
            for img in range(BC):
                for nch, (oy0, nyy) in enumerate(((0, 13), (13, 12))):
                    nc.scalar.activation(
                        h2_sb[:, (mch * BC + img) * 625 + oy0 * 25:
                              (mch * BC + img) * 625 + (oy0 + nyy) * 25],
                        ps2[(img, nch)][:, :nyy * 25], AF.Relu,
                        bias=b2_sb[:, mch:mch + 1])

        # ---- maxpool2 ----
        p2_sb = ca.tile([128, 2 * BC * 144], BF16, tag="p2")
        for mch in range(2):
            for img in range(BC):
                base = (mch * BC + img) * 625
                def h2v(ky, kx):
                    return _pv(h2_sb, 0, base + ky * 25 + kx,
                               [[50, 12], [2, 12]])
                dst = p2_sb[:, (mch * BC + img) * 144:(mch * BC + img + 1) * 144]
                d3 = dst.rearrange("p (a b) -> p a b", b=12)
                nc.vector.tensor_max(d3, h2v(0, 0), h2v(0, 1))
                for t in range(2, 9):
                    ky, kx = divmod(t, 3)
                    nc.vector.tensor_max(d3, d3, h2v(ky, kx))

        # ---- primarycaps conv (no relu) ----
        pc_sb = ca.tile([128, 2 * BC * 81], F32, tag="pc")
        for mch in range(2):
            psP = cps1.tile([128, 2 * 81], F32, tag="psP",
                            name=f"psP_{mch}")
            for tap in range(16):
                ky, kx = divmod(tap, 4)
                for kc in range(2):
                    lhs = wpt_sb[:, (tap * 2 + kc) * 256 + mch * 128:
                                 (tap * 2 + kc) * 256 + (mch + 1) * 128]
                    rhs = _pv(p2_sb, 0, kc * BC * 144 + ky * 12 + kx,
                              [[144, 2], [12, 9], [1, 9]])
                    nc.tensor.matmul(psP[:], lhs, rhs,
                                     start=(tap == 0 and kc == 0),
                                     stop=(tap == 15 and kc == 1))
            for img in range(BC):
                nc.vector.tensor_scalar_add(
                    pc_sb[:, (mch * BC + img) * 81:(mch * BC + img + 1) * 81],
                    psP[:, img * 81:(img + 1) * 81], bp_sb[:, mch:mch + 1])

        for mch in range(2):
            for img in range(BC):
                nc.sync.dma_start(
                    out=upc[img, mch * 128 * 81:(mch + 1) * 128 * 81]
                    .rearrange("(p f) -> p f", f=81),
                    in_=pc_sb[:, (mch * BC + img) * 81:(mch * BC + img + 1) * 81])

        # ---- squash -> u_loc (bf16) ----
        u_sb = ca.tile([128, BC * 21 * 8], F32, tag="usb")
        nc.vector.memset(u_sb[:], 0.0)
        for img in range(BC):
            nc.sync.dma_start(
                out=u_sb[:, img * 168:img * 168 + 160]
                .rearrange("p (c k) -> p c k", k=8),
                in_=_dap(upc, img * 20736, [[8, 128], [1024, 20], [1, 8]]))
            nc.sync.dma_start(
                out=u_sb[:32, img * 168 + 160:img * 168 + 168],
                in_=_dap(upc, img * 20736 + 20 * 1024, [[8, 32], [1, 8]]))
        n2 = ca.tile([128, BC * 21], F32, tag="sqn2")
        t1 = ca.tile([128, BC * 21], F32, tag="sqt1")
        r1 = ca.tile([128, BC * 21], F32, tag="sqr1")
        sq = ca.tile([128, BC * 168], F32, tag="sqsq")
        nc.scalar.activation(sq[:], u_sb[:], AF.Square)
        nc.vector.tensor_reduce(n2[:], sq[:].rearrange("p (c k) -> p c k", k=8),
                                AX.X, OP.add)
        nc.scalar.add(t1[:], n2[:], 1.0)
        nc.vector.reciprocal(t1[:], t1[:])
        nc.vector.tensor_scalar(t1[:], t1[:], -1.0, 1.0, OP.mult, OP.add)
        nc.vector.tensor_scalar_add(r1[:], n2[:], 1e-8)
        nc.scalar.activation(r1[:], r1[:], AF.Sqrt)
        nc.vector.reciprocal(r1[:], r1[:])
        nc.vector.tensor_mul(t1[:], t1[:], r1[:])
        u_bf = ca.tile([128, BC * 168], BF16, tag="ubf")
        nc.vector.tensor_mul(
            u_bf[:].rearrange("p (c k) -> p c k", k=8),
            u_sb[:].rearrange("p (c k) -> p c k", k=8),
            t1[:].rearrange("p (c one) -> p c one", one=1)
            .broadcast_to((128, BC * 21, 8)))
        for img in range(BC):
            nc.sync.dma_start(
                out=_dap(u_loc, img * 21504, [[8, 128], [1024, 21], [1, 8]]),
                in_=u_bf[:, img * 168:(img + 1) * 168]
                .rearrange("p (c k) -> p c k", k=8))

    # batch-shard -> i-shard via AllToAll (bf16 payload)
    nc.sync.dma_start(
        out=_dap(u_a2a, 0, [[5376, NCORES], [2688, BC], [1, 2688]]),
        in_=_dap(u_loc, 0, [[2688, NCORES], [21504, BC], [1, 2688]]))
    nc.gpsimd.collective_compute("AllToAll", OP.bypass, replica_groups=RG,
                                 ins=[u_a2a.opt()], outs=[u_mine.opt()])
    # u_mine as flat [16, 2688] bf16: b-major (core j's 2 images in order)

    # =========================================================
    # Phase B: routing (X resident bf16, whole-X DVE chunks)
    # =========================================================
    with (
        tc.tile_pool(name="rt", bufs=1) as rt,
        tc.tile_pool(name="rsm", bufs=1) as rsm,
        tc.tile_pool(name="rpv", bufs=1, space="PSUM") as rpv,
    ):
        rx_cm = tc.tile_pool(name="rx", bufs=1)
        rx = rx_cm.__enter__()
        X_sb = rx.tile([128, G * OD], BF16, tag="X")
        smat_sb = rt.tile([128, 16], BF16, tag="smt")
        nc.sync.dma_start(out=smat_sb[:], in_=T["smat"].ap())
        vrep = rt.tile([128, OD], BF16, tag="vrep")
        v_bf = rt.tile([16, OD], BF16, tag="vbf")
        vsum = rt.tile([16, OD], BF16, tag="vsum")

        def squash16(src, dst):
            """dst(bf16) = squash(src) over d; src [16, OD]."""
            qn2 = rsm.tile([16, O], F32, tag="q16a")
            qt = rsm.tile([16, O], F32, tag="q16b")
            qr = rsm.tile([16, O], F32, tag="q16c")
            qs = rsm.tile([16, OD], BF16, tag="q16d")
            nc.scalar.activation(qs[:], src, AF.Square)
            with nc.allow_low_precision(reason="squash norm accum"):
                nc.vector.tensor_reduce(
                    qn2[:], _pv(qs, 0, 0, [[1, O], [O, D]]), AX.X, OP.add)
            nc.scalar.add(qt[:], qn2[:], 1.0)
            nc.vector.reciprocal(qt[:], qt[:])
            nc.vector.tensor_scalar(qt[:], qt[:], -1.0, 1.0, OP.mult, OP.add)
            nc.vector.tensor_scalar_add(qr[:], qn2[:], 1e-8)
            nc.scalar.activation(qr[:], qr[:], AF.Sqrt)
            nc.vector.reciprocal(qr[:], qr[:])
            nc.vector.tensor_mul(qt[:], qt[:], qr[:])
            # d-major: dst[(d,o)] = src[(d,o)] * qt[o]
            nc.vector.tensor_mul(
                AP(tensor=dst.tensor, offset=dst.offset,
                   ap=[list(dst.ap[0]), [O, D], [1, O]]),
                AP(tensor=src.tensor, offset=src.offset,
                   ap=[list(src.ap[0]), [O, D], [1, O]]),
                _pv(qt, 0, 0, [[0, D], [1, O]]))

        def vrep_fill():
            for j in range(8):
                nc.sync.dma_start(out=vrep[j * 16:(j + 1) * 16, :],
                                  in_=v_bf[:])

        def v_iter(it, pvp, scale):
            """pvp psum [16,2048] -> AllReduce(bf16) -> squash -> v_bf."""
            vps = rsm.tile([16, OD], BF16, tag="vps")
            if scale != 1.0:
                nc.scalar.mul(vps[:], pvp[:, 0:OD], scale)
            else:
                nc.scalar.copy(vps[:], pvp[:, 0:OD])
            nc.sync.dma_start(out=v_in[it], in_=vps[:])
            nc.gpsimd.collective_compute(
                "AllReduce", OP.add, replica_groups=RG,
                ins=[v_in[it].opt()], outs=[v_out[it].opt()])
            nc.sync.dma_start(out=vsum[:], in_=v_out[it])
            squash16(vsum[:], v_bf[:])

        # ---- pass 0: build X (bf16, resident) + uniform-c v0 ----
        pvp = rpv.tile([16, 2048], F32, tag="pvp")
        with (
            tc.tile_pool(name="rtu", bufs=1) as rtu,
            tc.tile_pool(name="rws", bufs=6) as rws,
            tc.tile_pool(name="rpx", bufs=1, space="PSUM") as rpx,
        ):
            # u_mT[cap, b] = u_mine[b, cap]  (b innermost for ubd gather)
            nc.sync.dma_start(
                out=_dap(u_mT, 0, [[16, IPAD], [1, 16]]),
                in_=_dap(u_mine, 0, [[1, IPAD], [IPAD, 16]]))
            # block-diag u: ubd[c*8+k, g*128+c*16+b] = u_mT[(8g+c)*8+k, b]
            ubd = rtu.tile([64, G * 128], BF16, tag="ubd")
            nc.vector.memset(ubd[:], 0.0)
            for c in range(8):
                nc.sync.dma_start(
                    out=_pv(ubd, c * 8, c * 16, [["P", 8], [128, G], [1, 16]]),
                    in_=_dap(u_mT, 128 * c, [[16, 8], [1024, G], [1, 16]]))
            # dense u: ud[c*8+k, g*16+b] = u_mT[(8g+c)*8+k, b]
            ud = rtu.tile([64, G * 16], BF16, tag="ud")
            for c in range(8):
                nc.sync.dma_start(
                    out=_pv(ud, c * 8, 0, [["P", 8], [16, G], [1, 16]]),
                    in_=_dap(u_mT, 128 * c, [[16, 8], [1024, G], [1, 16]]))
            CK = ((0, 512), (512, 1024), (1024, 1536), (1536, OD))
            for g in range(G):
                wt = rws.tile([64, OD], BF16, tag="wt")
                nc.sync.dma_start(out=wt[:], in_=T["wrg"][g, :, :])
                lhs = ubd[:, g * 128:(g + 1) * 128]
                Xp = [rpx.tile([128, 512], F32, tag=f"Xp{j}",
                               name=f"Xp{g}_{j}") for j in range(4)]
                for j, (c0, c1) in enumerate(CK):
                    nc.tensor.matmul(Xp[j][:, :c1 - c0], lhs, wt[:, c0:c1],
                                     start=True, stop=True)
                for j, (c0, c1) in enumerate(CK):
                    if j < 2:
                        nc.scalar.copy(X_sb[:, g * OD + c0:g * OD + c1],
                                       Xp[j][:, :c1 - c0])
                    else:
                        nc.vector.tensor_copy(
                            out=X_sb[:, g * OD + c0:g * OD + c1],
                            in_=Xp[j][:, :c1 - c0])
                for (c0, c1) in CK:
                    nc.tensor.matmul(pvp[:, c0:c1],
                                     ud[:, g * 16:(g + 1) * 16],
                                     wt[:, c0:c1],
                                     start=(g == 0), stop=(g == G - 1),
                                     skip_group_check=True)
        v_iter(0, pvp, 1.0 / O)
        vrep_fill()

        # ---- passes 1, 2 ----
        b_sb = rt.tile([128, G * O], BF16, tag="blog")
        nch = (G + CH - 1) // CH
        rse_cm = tc.tile_pool(name="rse", bufs=2)
        rse = rse_cm.__enter__()
        rse1_cm = tc.tile_pool(name="rse1", bufs=1)
        rse1 = rse1_cm.__enter__()
        for it in (1, 2):
            pvp = rpv.tile([16, 2048], F32, tag="pvp")
            def chparts(cw):
                return [(nc.vector, 0, cw)]

            def stageA(ch):
                """tv = X * vrep for chunk ch."""
                g0 = ch * CH
                cw = min(CH, G - g0)
                tv = rse1.tile([128, CH * OD], BF16, tag="tvs",
                               name=f"tv{it}_{ch}")
                for eng, r0, rn in chparts(cw):
                    eng.tensor_mul(
                        _pv(tv, 0, r0 * OD, [[OD, rn], [1, OD]]),
                        _pv(X_sb, 0, (g0 + r0) * OD, [[OD, rn], [1, OD]]),
                        _pv(vrep, 0, 0, [[0, rn], [1, OD]]))
                return tv

            def stageB(ch, tv):
                """tree-reduce, logit update, exp(b - max) per group."""
                g0 = ch * CH
                cw = min(CH, G - g0)
                for eng, r0, rn in chparts(cw):
                    for hw in (8, 4, 2):
                        eng.tensor_add(
                            _pv(tv, 0, r0 * OD, [[OD, rn], [O, hw], [1, O]]),
                            _pv(tv, 0, r0 * OD, [[OD, rn], [O, hw], [1, O]]),
                            _pv(tv, 0, r0 * OD + hw * O,
                                [[OD, rn], [O, hw], [1, O]]))
                    if it == 1:
                        eng.tensor_add(
                            _pv(b_sb, 0, (g0 + r0) * O, [[O, rn], [1, O]]),
                            _pv(tv, 0, r0 * OD, [[OD, rn], [1, O]]),
                            _pv(tv, 0, r0 * OD + O, [[OD, rn], [1, O]]))
                    else:
                        db = rsm.tile([128, CH * O], BF16, tag="db")
                        eng.tensor_add(
                            _pv(db, 0, r0 * O, [[O, rn], [1, O]]),
                            _pv(tv, 0, r0 * OD, [[OD, rn], [1, O]]),
                            _pv(tv, 0, r0 * OD + O, [[OD, rn], [1, O]]))
                        eng.tensor_add(
                            _pv(b_sb, 0, (g0 + r0) * O, [[1, rn * O]]),
                            _pv(b_sb, 0, (g0 + r0) * O, [[1, rn * O]]),
                            _pv(db, 0, r0 * O, [[1, rn * O]]))
                b3 = _pv(b_sb, 0, g0 * O, [[O, cw], [1, O]])
                nm = rsm.tile([128, CH], BF16, tag="nm")
                nc.vector.tensor_reduce(_pv(nm, 0, 0, [[1, cw]]), b3,
                                        AX.X, OP.max, negate=True)
                eb = rse.tile([128, CH * O], BF16, tag="eb")
                s = rse.tile([128, CH], F32, tag="s")
                for gg in range(cw):
                    nc.scalar.activation(
                        _pv(eb, 0, gg * O, [[1, O]]),
                        _pv(b_sb, 0, (g0 + gg) * O, [[1, O]]),
                        AF.Exp, bias=_pv(nm, 0, gg, [[1, 1]]),
                        accum_out=_pv(s, 0, gg, [[1, 1]]))
                return eb, s

            def stageC(ch, eb, s):
                """normalize c, cx = X * c, pvp accumulation."""
                g0 = ch * CH
                cw = min(CH, G - g0)
                rs = rsm.tile([128, CH], BF16, tag="rs")
                with nc.allow_low_precision(reason="softmax recip bf16"):
                    nc.vector.reciprocal(_pv(rs, 0, 0, [[1, cw]]),
                                         _pv(s, 0, 0, [[1, cw]]))
                cn = rsm.tile([128, CH * O], BF16, tag="cn")
                nc.vector.tensor_mul(
                    _pv(cn, 0, 0, [[O, cw], [1, O]]),
                    _pv(eb, 0, 0, [[O, cw], [1, O]]),
                    _pv(rs, 0, 0, [[1, cw], [0, O]]))
                cx = rse.tile([128, CH * OD], BF16, tag="cxs",
                              name=f"cx{it}_{ch}")
                for eng, r0, rn in chparts(cw):
                    eng.tensor_mul(
                        _pv(cx, 0, r0 * OD, [[OD, rn], [O, D], [1, O]]),
                        _pv(X_sb, 0, (g0 + r0) * OD, [[OD, rn], [O, D], [1, O]]),
                        _pv(cn, 0, r0 * O, [[O, rn], [0, D], [1, O]]))
                for gg in range(cw):
                    glob = g0 + gg
                    for (c0, c1) in ((0, 512), (512, 1024), (1024, 1536),
                                     (1536, OD)):
                        nc.tensor.matmul(pvp[:, c0:c1], smat_sb[:],
                                         cx[:, gg * OD + c0:gg * OD + c1],
                                         start=(glob == 0),
                                         stop=(glob == G - 1),
                                         skip_group_check=True)

            # software pipeline: tv(k+1) issues while ACT runs exps(k)
            tv = stageA(0)
            pend = None
            for ch in range(nch):
                eb, s = stageB(ch, tv)
                if ch + 1 < nch:
                    tv = stageA(ch + 1)
                stageC(ch, eb, s)
            if it == 2:
                rse1_cm.__exit__(None, None, None)
                rse_cm.__exit__(None, None, None)
                rx_cm.__exit__(None, None, None)
            v_iter(it, pvp, 1.0)
            if it == 1:
                vrep_fill()

        v_od = rt.tile([16, OD], BF16, tag="vod")
        nc.vector.tensor_copy(
            out=_pv(v_od, 0, 0, [[D, O], [1, D]]),
            in_=_pv(v_bf, 0, 0, [[1, O], [O, D]]))
        nc.sync.dma_start(out=v2d.rearrange("(p f) -> p f", f=OD),
                          in_=v_od[:])

    # =========================================================
    # Phase C: caps conv + FC head
    # =========================================================
    with (
        tc.tile_pool(name="fcw", bufs=1) as fcw,
        tc.tile_pool(name="fcs", bufs=8) as fcs,
        tc.tile_pool(name="fcb", bufs=1) as fcb,
        tc.tile_pool(name="fca", bufs=1) as fca,
    ):
        caps3 = fca.tile([3, B * OD], BF16, tag="caps3")
        for kh in range(3):
            ln = B * OD - kh * D
            nc.sync.dma_start(
                out=caps3[kh:kh + 1, 0:ln],
                in_=v2d[kh * D:kh * D + ln].rearrange("(one f) -> one f", one=1))
        w3t_sb = fcw.tile([3, 768], BF16, tag="w3t")
        nc.sync.dma_start(out=w3t_sb[:], in_=T["w3T"].ap())
        b3_sb = fcw.tile([128, 2], F32, tag="b3s")
        nc.sync.dma_start(out=b3_sb[:].rearrange("c (m one) -> c m one", one=1),
                          in_=T["b3c"].ap().rearrange("m c one -> c m one"))
        fb1_sb = fcw.tile([16, 512], F32, tag="fb1")
        nc.sync.dma_start(out=fb1_sb[:], in_=T["fb1r"].ap())
        fb2_sb = fcw.tile([128, 32], F32, tag="fb2")
        nc.sync.dma_start(out=fb2_sb[:], in_=T["fb2T"].ap())
        fb3_sb = fcw.tile([16, 102], F32, tag="fb3")
        nc.sync.dma_start(out=fb3_sb[:], in_=T["fb3r"].ap())

        with tc.tile_pool(name="fp1", bufs=2, space="PSUM") as fp1:
            h3_sb = fca.tile([128, 2 * B * 350], BF16, tag="h3")
            for mch in range(2):
                for b in range(B):
                    ps = fp1.tile([128, 512], F32, tag="ps3")
                    for kw in range(3):
                        rhs = _pv(caps3, 0, b * OD + kw,
                                  [["P", 3], [32, 50], [2, 7]])
                        nc.tensor.matmul(
                            ps[:, :350],
                            w3t_sb[:, (kw * 2 + mch) * 128:
                                   (kw * 2 + mch + 1) * 128],
                            rhs, start=(kw == 0), stop=(kw == 2))
                    nc.scalar.activation(
                        h3_sb[:, mch * B * 350 + b * 350:
                              mch * B * 350 + (b + 1) * 350],
                        ps[:, :350], AF.Relu, bias=b3_sb[:, mch:mch + 1])
            p3_sb = fca.tile([128, 2 * B * 72], BF16, tag="p3")
            for mch in range(2):
                eng = nc.vector
                def h3v(ky, kx):
                    return _pv(h3_sb, 0, mch * B * 350 + ky * 7 + kx,
                               [[350, B], [14, 24], [2, 3]])
                dst = _pv(p3_sb, 0, mch * B * 72, [[1, B], [48, 24], [16, 3]])
                eng.tensor_max(dst, h3v(0, 0), h3v(0, 1))
                for t in range(2, 9):
                    ky, kx = divmod(t, 3)
                    eng.tensor_max(dst, dst, h3v(ky, kx))
            # ---- FC1 (output-sharded, 512 cols); lhsT chunks are
            # p3_sb slices directly: chunk kt=(mch,pos) -> [128 ch, 16 b]
            psf = fp1.tile([16, 512], F32, tag="psf")
            for blk in range(18):
                fwt = fcs.tile([128, 8 * 512], BF16, tag="fwt")
                nc.sync.dma_start(
                    out=fwt[:].rearrange("p (t f) -> p t f", f=512),
                    in_=T["fw1T"][blk, :, :].rearrange("p (t f) -> p t f",
                                                       f=512))
                for sub in range(8):
                    kt = blk * 8 + sub
                    nc.tensor.matmul(psf[:],
                                     p3_sb[:, kt * B:(kt + 1) * B],
                                     fwt[:, sub * 512:(sub + 1) * 512],
                                     start=(kt == 0), stop=(kt == 143))
            f1bf = fca.tile([16, 512], BF16, tag="f1bf")
            r1f = fca.tile([16, 512], F32, tag="r1f")
            nc.vector.tensor_add(r1f[:], psf[:], fb1_sb[:])
            nc.scalar.activation(f1bf[:], r1f[:], AF.Relu)
            nc.sync.dma_start(out=_dap(f1T, 0, [[1, 16], [16, 512]]),
                              in_=f1bf[:])

        # ---- FC2 (input-sharded) + AllReduce ----
        f2l = fca.tile([128, 4 * B], BF16, tag="f2l")
        nc.sync.dma_start(
            out=f2l[:].rearrange("p (c b) -> p c b", b=B),
            in_=_dap(f1T, 0, [[16, 128], [2048, 4], [1, 16]]))
        fw2_sb = fcb.tile([128, 4 * 4096], BF16, tag="fw2")
        for cc in range(4):
            nc.sync.dma_start(out=fw2_sb[:, cc * 4096:(cc + 1) * 4096],
                              in_=T["fw2T"][cc, :, :])
        with tc.tile_pool(name="fp2", bufs=1, space="PSUM") as fp2:
            z2p = fp2.tile([16, 4096], F32, tag="z2p")
            for cc in range(4):
                lhs = f2l[:, cc * B:(cc + 1) * B]
                for j in range(8):
                    nc.tensor.matmul(
                        z2p[:, j * 512:(j + 1) * 512], lhs,
                        fw2_sb[:, cc * 4096 + j * 512:cc * 4096 + (j + 1) * 512],
                        start=(cc == 0), stop=(cc == 3))
            z2s = fca.tile([16, 4096], BF16, tag="z2s")
            nc.scalar.copy(z2s[:, 0:2048], z2p[:, 0:2048])
            nc.vector.tensor_copy(out=z2s[:, 2048:4096], in_=z2p[:, 2048:4096])
            # store transposed: z2in flat[(u, b)] = z2s[b, u]
            nc.sync.dma_start(out=_dap(z2in, 0, [[1, 16], [16, 4096]]),
                              in_=z2s[:])
        nc.gpsimd.collective_compute("AllReduce", OP.add, replica_groups=RG,
                                     ins=[z2in.opt()], outs=[z2out.opt()])
        # reload: z2T[p, cc, b] = z2out_flat[(cc*128+p)*16 + b]
        z2T = fca.tile([128, 32 * B], BF16, tag="z2T")
        nc.sync.dma_start(
            out=z2T[:].rearrange("p (c b) -> p c b", b=B),
            in_=_dap(z2out, 0, [[16, 128], [2048, 32], [1, 16]]))
        f3l = fca.tile([128, 32 * B], BF16, tag="f3l")
        nc.vector.tensor_add(
            z2T[:].rearrange("p (c b) -> p c b", b=B),
            z2T[:].rearrange("p (c b) -> p c b", b=B),
            _pv(fb2_sb, 0, 0, [[1, 32], [0, B]]))
        nc.scalar.activation(f3l[:], z2T[:], AF.Relu)

        # ---- FC3 (full, every core) ----
        fw3_sb = fcb.tile([128, 32 * 102], BF16, tag="fw3")
        nc.sync.dma_start(
            out=fw3_sb[:].rearrange("p (t f) -> p t f", f=102),
            in_=T["fw3T"].ap().rearrange("t p f -> p t f"))
        with tc.tile_pool(name="fp3", bufs=1, space="PSUM") as fp3:
            ps3f = fp3.tile([16, 512], F32, tag="ps3f")
            for cc in range(32):
                nc.tensor.matmul(ps3f[:, :102], f3l[:, cc * B:(cc + 1) * B],
                                 fw3_sb[:, cc * 102:(cc + 1) * 102],
                                 start=(cc == 0), stop=(cc == 31))
            res3 = fca.tile([16, 102], F32, tag="res3")
            nc.vector.tensor_add(res3[:], ps3f[:, :102], fb3_sb[:])
            nc.sync.dma_start(out=out_t[:, :], in_=res3[:])


def _prep_inputs(inputs):
    import ml_dtypes
    bf = ml_dtypes.bfloat16
    x = np.ascontiguousarray(inputs["x"], dtype=np.float32)
    w1, b1 = inputs["w1"], inputs["b1"]
    w2, b2 = inputs["w2"], inputs["b2"]
    wp, bp = inputs["wp"], inputs["bp"]
    Wcap = inputs["Wcap"]
    w3, b3 = inputs["w3"], inputs["b3"]
    fw1, fb1 = inputs["fw1"], inputs["fb1"]
    fw2, fb2 = inputs["fw2"], inputs["fb2"]
    fw3, fb3 = inputs["fw3"], inputs["fb3"]

    s = x.strides
    xw = as_strided(x, shape=(B, 3, 11, 11, 51, 51),
                    strides=(s[0], s[1], s[2], s[3], 4 * s[2], 4 * s[3]))
    xcols = np.ascontiguousarray(xw, dtype=bf).reshape(B, 363, 2601)

    w1T = np.ascontiguousarray(np.asarray(w1).reshape(96, 363).T, dtype=bf)
    w2T = np.ascontiguousarray(np.asarray(w2).transpose(2, 3, 1, 0),
                               dtype=bf).reshape(25, 96, 256)
    wpT = np.ascontiguousarray(np.asarray(wp).transpose(2, 3, 1, 0),
                               dtype=bf).reshape(16, 2, 128, 256)
    w3T = np.ascontiguousarray(
        np.asarray(w3).reshape(256, 9).T.reshape(3, 3, 256),
        dtype=bf).reshape(3, 768)

    Wp = np.zeros((O, IPAD, D, 8), np.float32)
    Wp[:, :ITOT] = np.asarray(Wcap)
    # d-major od columns: col = d*O + o
    wrg_all = np.ascontiguousarray(
        Wp.reshape(O, NCORES, G, 8, D, 8).transpose(1, 2, 3, 5, 4, 0),
        dtype=bf).reshape(NCORES, G, 64, OD)

    fw1 = np.asarray(fw1)
    fw2 = np.asarray(fw2)
    fw3 = np.asarray(fw3)
    # chunk kt = (mch, pos): lhsT = p3_sb[:, kt*16:(kt+1)*16] whose
    # partition p maps to f-index (mch*128 + p)*72 + pos.
    # fw1T[r][blk, p, sub*512+f] with kt = blk*8+sub = mch*72+pos
    fw1T_all = np.ascontiguousarray(
        fw1.reshape(NCORES, 512, 18432).transpose(0, 2, 1)
        .reshape(NCORES, 2, 128, 72, 512).transpose(0, 1, 3, 2, 4)
        .reshape(NCORES, 18, 8, 128, 512).transpose(0, 1, 3, 2, 4),
        dtype=bf).reshape(NCORES, 18, 128, 8 * 512)
    # fw2 input-shard: [r] -> fw2[:, 512r:512(r+1)].T -> [4, 128, 4096]
    fw2in_all = np.ascontiguousarray(
        fw2.T.reshape(NCORES, 512, 4096), dtype=bf
    ).reshape(NCORES, 4, 128, 4096)
    fw3T = np.ascontiguousarray(fw3.T.reshape(32, 128, 102), dtype=bf)
    fb2T = np.ascontiguousarray(
        np.asarray(fb2, np.float32).reshape(32, 128).T)

    shared = dict(
        w1T=w1T, b1c=np.asarray(b1, np.float32).reshape(96, 1),
        w2T=w2T, b2c=np.asarray(b2, np.float32).reshape(2, 128, 1),
        wpT=wpT, bpc=np.asarray(bp, np.float32).reshape(2, 128, 1),
        w3T=w3T, b3c=np.asarray(b3, np.float32).reshape(2, 128, 1),
        smat=np.ascontiguousarray(
            np.tile(np.eye(16, dtype=np.float32), (8, 1)), dtype=bf),
        fw3T=fw3T, fb2T=fb2T,
        fb3r=np.ascontiguousarray(
            np.tile(np.asarray(fb3, np.float32).reshape(1, 102), (16, 1))))
    in_maps = []
    for r in range(NCORES):
        m = dict(shared)
        m["xcols"] = np.ascontiguousarray(xcols[2 * r:2 * r + 2])
        m["wrg"] = np.ascontiguousarray(wrg_all[r])
        m["fw1T"] = np.ascontiguousarray(fw1T_all[r])
        m["fw2T"] = np.ascontiguousarray(fw2in_all[r])
        m["fb1r"] = np.ascontiguousarray(
            np.tile(np.asarray(fb1, np.float32)[512 * r:512 * (r + 1)]
                    .reshape(1, 512), (16, 1)))
        in_maps.append(m)
    return in_maps


def kernel(**inputs):
    if "nc" not in _CACHE:
        _CACHE["nc"] = build_program()
    in_maps = _prep_inputs(inputs)
    last_err = None
    for attempt in range(3):
        try:
            res = run_bass_kernel_spmd(_CACHE["nc"], in_maps,
                                       list(range(NCORES)))
            _CACHE["last_exec_ns"] = res.exec_time_ns
            return np.asarray(res.results[0]["out"], dtype=np.float32)
        except Exception as err:  # transient device-unrecoverable states
            last_err = err
            import time as _time
            _time.sleep(20 * (attempt + 1))
    raise last_err


# revision 4
# speedup vs baseline: 1.1369x; 1.1369x over previous
"""CapsAlexNet (FLOWER102) forward pass on 8 Trainium2 NeuronCores — v2.

Sharding (same global structure as v1, heavily bf16 + restructured):
  - conv stack: data-parallel over batch (2 images/core); conv1 via host
    im2col; all matmuls bf16 (fp32 PSUM accumulate).
  - capsule routing: capsule dim sharded 8 ways (AllToAll from batch-shard
    to i-shard). x_hat (X) computed ONCE in bf16 and kept resident in SBUF
    (17.5MB); the two logit/softmax passes run whole-X DVE ops in chunks of
    CH groups. AllReduce of [16,1632] bf16 per routing iteration (3 total).
  - caps conv computed fully on every core (v is global after AllReduce).
  - FC head: FC1 output-sharded (512 cols/core, bf16 weights streamed),
    FC2 input-sharded with a single AllReduce of the pre-activation,
    FC3 computed fully on every core.
  Collectives: AllToAll + 3x AllReduce + 1x AllReduce = 5.
"""

import numpy as np
from numpy.lib.stride_tricks import as_strided

import concourse.bass as bass
import concourse.mybir as mybir
import concourse.tile as tile
from concourse import bacc
from concourse.ap import AP
from concourse.bass_utils import run_bass_kernel_spmd

F32 = mybir.dt.float32
BF16 = mybir.dt.bfloat16
AX = mybir.AxisListType
OP = mybir.AluOpType
AF = mybir.ActivationFunctionType

NCORES = 8
B = 16
BC = 2           # images per core
O = 102
D = 16
OD = O * D       # 1632
ITOT = 2592
IPAD = 2688
ILOC = IPAD // NCORES   # 336
G = ILOC // 8           # 42 groups of 8 caps
CH = 4                  # groups per DVE chunk in routing passes
RG = [list(range(NCORES))]

_CACHE = {}


def _dap(a, offset, dims):
    """Manual AP into a dram-pool tile (which is itself an AP)."""
    return AP(tensor=a.tensor, offset=a.offset + offset,
              ap=[list(d) for d in dims])


def _pv(t, part0, free0, dims):
    """AP into SBUF tile t at (partition part0, free offset free0).

    dims: list of [step, count] free dims; prepend ("P", n) to set the
    partition count (default: full tile partitions).
    """
    base = t[:]
    fs = base.ap[0][0]          # partition stride == free size
    npart = dims[0][1] if dims[0][0] == "P" else base.ap[0][1]
    rest = dims[1:] if dims[0][0] == "P" else dims
    return AP(tensor=base.tensor, offset=base.offset + part0 * fs + free0,
              ap=[[fs, npart]] + [list(d) for d in rest])


def build_program():
    nc = bacc.Bacc("TRN2", target_bir_lowering=False, debug=False,
                   num_devices=NCORES)

    def din(name, shape, dt=F32):
        return nc.declare_dram_parameter(name, list(shape), dt, isOutput=False)

    T = dict(
        xcols=din("xcols", [BC, 363, 2601], BF16),
        w1T=din("w1T", [363, 96], BF16), b1c=din("b1c", [96, 1]),
        w2T=din("w2T", [25, 96, 256], BF16), b2c=din("b2c", [2, 128, 1]),
        wpT=din("wpT", [16, 2, 128, 256], BF16), bpc=din("bpc", [2, 128, 1]),
        wrg=din("wrg", [G, 64, OD], BF16),
        smat=din("smat", [128, 16], BF16),
        w3T=din("w3T", [3, 768], BF16), b3c=din("b3c", [2, 128, 1]),
        fw1T=din("fw1T", [18, 128, 8 * 512], BF16),
        fb1r=din("fb1r", [16, 512]),
        fw2T=din("fw2T", [4, 128, 4096], BF16),
        fb2T=din("fb2T", [128, 32]),
        fw3T=din("fw3T", [32, 128, 102], BF16),
        fb3r=din("fb3r", [16, 102]),
    )
    T["out_t"] = nc.declare_dram_parameter("out", [B, O], F32, isOutput=True)

    with tile.TileContext(nc) as tc:
        with tc.tile_pool(name="dram", bufs=1, space="DRAM") as dram:
            _build_body(nc, tc, dram, T)
    nc.finalize()
    return nc


def _build_body(nc, tc, dram, T):
    out_t = T["out_t"]

    # ---------------- DRAM scratch ----------------
    upc = dram.tile([BC, 20736], F32, tag="upc")
    u_loc = dram.tile([BC, IPAD * 8], BF16, tag="uloc")
    u_a2a = dram.tile([NCORES, BC, ILOC * 8], BF16, tag="ua2a")
    u_mine = dram.tile([NCORES, BC, ILOC * 8], BF16, tag="umine")
    u_mT = dram.tile([ILOC * 8, B], BF16, tag="umT")
    v_in = [dram.tile([16, OD], BF16, tag=f"vin{i}", name=f"vin{i}")
            for i in range(3)]
    v_out = [dram.tile([16, OD], BF16, tag=f"vout{i}", name=f"vout{i}")
             for i in range(3)]
    v2d = dram.tile([B * OD], BF16, tag="v2d")
    f1T = dram.tile([512, B], BF16, tag="f1T")
    z2in = dram.tile([16, 4096], BF16, tag="z2in")
    z2out = dram.tile([16, 4096], BF16, tag="z2out")

    # =========================================================
    # Phase A: conv stack (2 images, bf16)
    # =========================================================
    with (
        tc.tile_pool(name="caw", bufs=1) as cw,
        tc.tile_pool(name="cact", bufs=1) as ca,
        tc.tile_pool(name="cps", bufs=2, space="PSUM") as cps,
        tc.tile_pool(name="cps1", bufs=1, space="PSUM") as cps1,
    ):
        # conv1 inputs + weights first (everything else overlaps conv1)
        xc_sb = ca.tile([128, BC * 3 * 2601], BF16, tag="xc")
        for img in range(BC):
            for kt in range(3):
                rows = 128 if kt < 2 else 107
                c0 = (img * 3 + kt) * 2601
                nc.sync.dma_start(out=xc_sb[:rows, c0:c0 + 2601],
                                  in_=T["xcols"][img, kt * 128:kt * 128 + rows, :])
        w1t_sb = cw.tile([128, 3 * 96], BF16, tag="w1t")
        for kt in range(3):
            rows = 128 if kt < 2 else 107
            nc.sync.dma_start(out=w1t_sb[:rows, kt * 96:(kt + 1) * 96],
                              in_=T["w1T"][kt * 128:kt * 128 + rows, :])
        b1_sb = cw.tile([96, 1], F32, tag="b1s")
        nc.sync.dma_start(out=b1_sb[:], in_=T["b1c"][:, :])
        w2t_sb = cw.tile([96, 25 * 256], BF16, tag="w2t")
        nc.sync.dma_start(out=w2t_sb[:].rearrange("p (t o) -> p t o", o=256),
                          in_=T["w2T"].ap().rearrange("t c o -> c t o"))
        wpt_sb = cw.tile([128, 32 * 256], BF16, tag="wpt")
        nc.sync.dma_start(
            out=wpt_sb[:].rearrange("p (t k o) -> p t k o", k=2, o=256),
            in_=T["wpT"].ap().rearrange("t k c o -> c t k o"))
        b2_sb = cw.tile([128, 2], F32, tag="b2s")
        nc.sync.dma_start(out=b2_sb[:].rearrange("c (m one) -> c m one", one=1),
                          in_=T["b2c"].ap().rearrange("m c one -> c m one"))
        bp_sb = cw.tile([128, 2], F32, tag="bps")
        nc.sync.dma_start(out=bp_sb[:].rearrange("c (m one) -> c m one", one=1),
                          in_=T["bpc"].ap().rearrange("m c one -> c m one"))

        # ---- conv1 + relu ----
        h1_sb = ca.tile([96, BC * 2601], BF16, tag="h1")
        for img in range(BC):
            for (n0, n1) in ((0, 512), (512, 1024), (1024, 1536),
                             (1536, 2048), (2048, 2560), (2560, 2601)):
                ps = cps.tile([96, 512], F32, tag="ps1")
                for kt in range(3):
                    rows = 128 if kt < 2 else 107
                    c0 = (img * 3 + kt) * 2601
                    nc.tensor.matmul(ps[:, :n1 - n0],
                                     w1t_sb[:rows, kt * 96:(kt + 1) * 96],
                                     xc_sb[:rows, c0 + n0:c0 + n1],
                                     start=(kt == 0), stop=(kt == 2))
                nc.scalar.activation(h1_sb[:, img * 2601 + n0:img * 2601 + n1],
                                     ps[:, :n1 - n0], AF.Relu, bias=b1_sb[:, 0:1])

        # ---- maxpool1 -> padded conv2 input ----
        p1p_sb = ca.tile([96, BC * 841], BF16, tag="p1p")
        nc.vector.memset(p1p_sb[:], 0.0)
        for img in range(BC):
            eng = nc.vector
            def h1v(ky, kx):
                return _pv(h1_sb, 0, img * 2601 + ky * 51 + kx,
                           [[102, 25], [2, 25]])
            dst = _pv(p1p_sb, 0, img * 841 + 2 * 29 + 2, [[29, 25], [1, 25]])
            eng.tensor_max(dst, h1v(0, 0), h1v(0, 1))
            for t in range(2, 9):
                ky, kx = divmod(t, 3)
                eng.tensor_max(dst, dst, h1v(ky, kx))

        # ---- conv2 + relu ----
        h2_sb = ca.tile([128, 2 * BC * 625], BF16, tag="h2")
        for mch in range(2):
            ps2 = {}
            for img in range(BC):
                for nch in range(2):
                    ps2[(img, nch)] = cps1.tile(
                        [128, 512], F32, tag=f"ps2_{img}_{nch}",
                        name=f"ps2_{mch}_{img}_{nch}")
            for tap in range(25):
                ky, kx = divmod(tap, 5)
                lhs = w2t_sb[:, tap * 256 + mch * 128:tap * 256 + (mch + 1) * 128]
                for img in range(BC):
                    for nch, (oy0, nyy) in enumerate(((0, 13), (13, 12))):
                        rhs = _pv(p1p_sb, 0,
                                  img * 841 + (oy0 + ky) * 29 + kx,
                                  [[29, nyy], [1, 25]])
                        nc.tensor.matmul(ps2[(img, nch)][:, :nyy * 25], lhs,
                                         rhs, start=(tap == 0), stop=(tap == 24))
            for img in range(BC):
                for nch, (oy0, nyy) in enumerate(((0, 13), (13, 12))):
                    nc.scalar.activation(
                        h2_sb[:, (mch * BC + img) * 625 + oy0 * 25:
                              (mch * BC + img) * 625 + (oy0 + nyy) * 25],
                        ps2[(img, nch)][:, :nyy * 25], AF.Relu,
                        bias=b2_sb[:, mch:mch + 1])

        # ---- maxpool2 ----
        p2_sb = ca.tile([128, 2 * BC * 144], BF16, tag="p2")
        for mch in range(2):
            for img in range(BC):
                base = (mch * BC + img) * 625
                def h2v(ky, kx):
                    return _pv(h2_sb, 0, base + ky * 25 + kx,
                               [[50, 12], [2, 12]])
                dst = p2_sb[:, (mch * BC + img) * 144:(mch * BC + img + 1) * 144]
                d3 = dst.rearrange("p (a b) -> p a b", b=12)
                nc.vector.tensor_max(d3, h2v(0, 0), h2v(0, 1))
                for t in range(2, 9):
                    ky, kx = divmod(t, 3)
                    nc.vector.tensor_max(d3, d3, h2v(ky, kx))

        # ---- primarycaps conv (no relu) ----
        pc_sb = ca.tile([128, 2 * BC * 81], F32, tag="pc")
        for mch in range(2):
            psP = cps1.tile([128, 2 * 81], F32, tag="psP",
                            name=f"psP_{mch}")
            for tap in range(16):
                ky, kx = divmod(tap, 4)
                for kc in range(2):
                    lhs = wpt_sb[:, (tap * 2 + kc) * 256 + mch * 128:
                                 (tap * 2 + kc) * 256 + (mch + 1) * 128]
                    rhs = _pv(p2_sb, 0, kc * BC * 144 + ky * 12 + kx,
                              [[144, 2], [12, 9], [1, 9]])
                    nc.tensor.matmul(psP[:], lhs, rhs,
                                     start=(tap == 0 and kc == 0),
                                     stop=(tap == 15 and kc == 1))
            for img in range(BC):
                nc.vector.tensor_scalar_add(
                    pc_sb[:, (mch * BC + img) * 81:(mch * BC + img + 1) * 81],
                    psP[:, img * 81:(img + 1) * 81], bp_sb[:, mch:mch + 1])

        for mch in range(2):
            for img in range(BC):
                nc.sync.dma_start(
                    out=upc[img, mch * 128 * 81:(mch + 1) * 128 * 81]
                    .rearrange("(p f) -> p f", f=81),
                    in_=pc_sb[:, (mch * BC + img) * 81:(mch * BC + img + 1) * 81])

        # ---- squash -> u_loc (bf16) ----
        u_sb = ca.tile([128, BC * 21 * 8], F32, tag="usb")
        nc.vector.memset(u_sb[:], 0.0)
        for img in range(BC):
            nc.sync.dma_start(
                out=u_sb[:, img * 168:img * 168 + 160]
                .rearrange("p (c k) -> p c k", k=8),
                in_=_dap(upc, img * 20736, [[8, 128], [1024, 20], [1, 8]]))
            nc.sync.dma_start(
                out=u_sb[:32, img * 168 + 160:img * 168 + 168],
                in_=_dap(upc, img * 20736 + 20 * 1024, [[8, 32], [1, 8]]))
        n2 = ca.tile([128, BC * 21], F32, tag="sqn2")
        t1 = ca.tile([128, BC * 21], F32, tag="sqt1")
        r1 = ca.tile([128, BC * 21], F32, tag="sqr1")
        sq = ca.tile([128, BC * 168], F32, tag="sqsq")
        nc.scalar.activation(sq[:], u_sb[:], AF.Square)
        nc.vector.tensor_reduce(n2[:], sq[:].rearrange("p (c k) -> p c k", k=8),
                                AX.X, OP.add)
        nc.scalar.add(t1[:], n2[:], 1.0)
        nc.vector.reciprocal(t1[:], t1[:])
        nc.vector.tensor_scalar(t1[:], t1[:], -1.0, 1.0, OP.mult, OP.add)
        nc.vector.tensor_scalar_add(r1[:], n2[:], 1e-8)
        nc.scalar.activation(r1[:], r1[:], AF.Sqrt)
        nc.vector.reciprocal(r1[:], r1[:])
        nc.vector.tensor_mul(t1[:], t1[:], r1[:])
        u_bf = ca.tile([128, BC * 168], BF16, tag="ubf")
        nc.vector.tensor_mul(
            u_bf[:].rearrange("p (c k) -> p c k", k=8),
            u_sb[:].rearrange("p (c k) -> p c k", k=8),
            t1[:].rearrange("p (c one) -> p c one", one=1)
            .broadcast_to((128, BC * 21, 8)))
        for img in range(BC):
            nc.sync.dma_start(
                out=_dap(u_loc, img * 21504, [[8, 128], [1024, 21], [1, 8]]),
                in_=u_bf[:, img * 168:(img + 1) * 168]
                .rearrange("p (c k) -> p c k", k=8))

    # batch-shard -> i-shard via AllToAll (bf16 payload)
    nc.sync.dma_start(
        out=_dap(u_a2a, 0, [[5376, NCORES], [2688, BC], [1, 2688]]),
        in_=_dap(u_loc, 0, [[2688, NCORES], [21504, BC], [1, 2688]]))
    nc.gpsimd.collective_compute("AllToAll", OP.bypass, replica_groups=RG,
                                 ins=[u_a2a.opt()], outs=[u_mine.opt()])
    # u_mine as flat [16, 2688] bf16: b-major (core j's 2 images in order)

    # =========================================================
    # Phase B: routing (X resident bf16, whole-X DVE chunks)
    # =========================================================
    with (
        tc.tile_pool(name="rt", bufs=1) as rt,
        tc.tile_pool(name="rsm", bufs=1) as rsm,
        tc.tile_pool(name="rpv", bufs=1, space="PSUM") as rpv,
    ):
        rx_cm = tc.tile_pool(name="rx", bufs=1)
        rx = rx_cm.__enter__()
        X_sb = rx.tile([128, G * OD], BF16, tag="X")
        smat_sb = rt.tile([128, 16], BF16, tag="smt")
        nc.sync.dma_start(out=smat_sb[:], in_=T["smat"].ap())
        vrep = rt.tile([128, OD], BF16, tag="vrep")
        v_bf = rt.tile([16, OD], BF16, tag="vbf")
        vsum = rt.tile([16, OD], BF16, tag="vsum")

        def squash16(src, dst):
            """dst(bf16) = squash(src) over d; src [16, OD]."""
            qn2 = rsm.tile([16, O], F32, tag="q16a")
            qt = rsm.tile([16, O], F32, tag="q16b")
            qr = rsm.tile([16, O], F32, tag="q16c")
            qs = rsm.tile([16, OD], BF16, tag="q16d")
            nc.scalar.activation(qs[:], src, AF.Square)
            with nc.allow_low_precision(reason="squash norm accum"):
                nc.vector.tensor_reduce(
                    qn2[:], _pv(qs, 0, 0, [[1, O], [O, D]]), AX.X, OP.add)
            nc.scalar.add(qt[:], qn2[:], 1.0)
            nc.vector.reciprocal(qt[:], qt[:])
            nc.vector.tensor_scalar(qt[:], qt[:], -1.0, 1.0, OP.mult, OP.add)
            nc.vector.tensor_scalar_add(qr[:], qn2[:], 1e-8)
            nc.scalar.activation(qr[:], qr[:], AF.Sqrt)
            nc.vector.reciprocal(qr[:], qr[:])
            nc.vector.tensor_mul(qt[:], qt[:], qr[:])
            # d-major: dst[(d,o)] = src[(d,o)] * qt[o]
            nc.vector.tensor_mul(
                AP(tensor=dst.tensor, offset=dst.offset,
                   ap=[list(dst.ap[0]), [O, D], [1, O]]),
                AP(tensor=src.tensor, offset=src.offset,
                   ap=[list(src.ap[0]), [O, D], [1, O]]),
                _pv(qt, 0, 0, [[0, D], [1, O]]))

        def vrep_fill():
            for j in range(8):
                nc.sync.dma_start(out=vrep[j * 16:(j + 1) * 16, :],
                                  in_=v_bf[:])

        def v_iter(it, pvp, scale):
            """pvp psum [16,2048] -> AllReduce(bf16) -> squash -> v_bf."""
            vps = rsm.tile([16, OD], BF16, tag="vps")
            if scale != 1.0:
                nc.scalar.mul(vps[:], pvp[:, 0:OD], scale)
            else:
                nc.scalar.copy(vps[:], pvp[:, 0:OD])
            nc.sync.dma_start(out=v_in[it], in_=vps[:])
            nc.gpsimd.collective_compute(
                "AllReduce", OP.add, replica_groups=RG,
                ins=[v_in[it].opt()], outs=[v_out[it].opt()])
            nc.sync.dma_start(out=vsum[:], in_=v_out[it])
            squash16(vsum[:], v_bf[:])

        # ---- pass 0: build X (bf16, resident) + uniform-c v0 ----
        pvp = rpv.tile([16, 2048], F32, tag="pvp")
        with (
            tc.tile_pool(name="rtu", bufs=1) as rtu,
            tc.tile_pool(name="rws", bufs=6) as rws,
            tc.tile_pool(name="rpx", bufs=1, space="PSUM") as rpx,
        ):
            # u_mT[cap, b] = u_mine[b, cap]  (b innermost for ubd gather)
            nc.sync.dma_start(
                out=_dap(u_mT, 0, [[16, IPAD], [1, 16]]),
                in_=_dap(u_mine, 0, [[1, IPAD], [IPAD, 16]]))
            # block-diag u: ubd[c*8+k, g*128+c*16+b] = u_mT[(8g+c)*8+k, b]
            ubd = rtu.tile([64, G * 128], BF16, tag="ubd")
            nc.vector.memset(ubd[:], 0.0)
            for c in range(8):
                nc.sync.dma_start(
                    out=_pv(ubd, c * 8, c * 16, [["P", 8], [128, G], [1, 16]]),
                    in_=_dap(u_mT, 128 * c, [[16, 8], [1024, G], [1, 16]]))
            # dense u: ud[c*8+k, g*16+b] = u_mT[(8g+c)*8+k, b]
            ud = rtu.tile([64, G * 16], BF16, tag="ud")
            for c in range(8):
                nc.sync.dma_start(
                    out=_pv(ud, c * 8, 0, [["P", 8], [16, G], [1, 16]]),
                    in_=_dap(u_mT, 128 * c, [[16, 8], [1024, G], [1, 16]]))
            CK = ((0, 512), (512, 1024), (1024, 1536), (1536, OD))
            for g in range(G):
                wt = rws.tile([64, OD], BF16, tag="wt")
                nc.sync.dma_start(out=wt[:], in_=T["wrg"][g, :, :])
                lhs = ubd[:, g * 128:(g + 1) * 128]
                Xp = [rpx.tile([128, 512], F32, tag=f"Xp{j}",
                               name=f"Xp{g}_{j}") for j in range(4)]
                for j, (c0, c1) in enumerate(CK):
                    nc.tensor.matmul(Xp[j][:, :c1 - c0], lhs, wt[:, c0:c1],
                                     start=True, stop=True)
                for j, (c0, c1) in enumerate(CK):
                    if j < 2:
                        nc.scalar.copy(X_sb[:, g * OD + c0:g * OD + c1],
                                       Xp[j][:, :c1 - c0])
                    else:
                        nc.vector.tensor_copy(
                            out=X_sb[:, g * OD + c0:g * OD + c1],
                            in_=Xp[j][:, :c1 - c0])
                for (c0, c1) in CK:
                    nc.tensor.matmul(pvp[:, c0:c1],
                                     ud[:, g * 16:(g + 1) * 16],
                                     wt[:, c0:c1],
                                     start=(g == 0), stop=(g == G - 1),
                                     skip_group_check=True)
        v_iter(0, pvp, 1.0 / O)
        vrep_fill()

        # ---- passes 1, 2 ----
        b_sb = rt.tile([128, G * O], BF16, tag="blog")
        nch = (G + CH - 1) // CH
        rse_cm = tc.tile_pool(name="rse", bufs=2)
        rse = rse_cm.__enter__()
        rse1_cm = tc.tile_pool(name="rse1", bufs=1)
        rse1 = rse1_cm.__enter__()
        for it in (1, 2):
            pvp = rpv.tile([16, 2048], F32, tag="pvp")
            def chparts(cw):
                return [(nc.vector, 0, cw)]

            def stageA(ch):
                """tv = X * vrep for chunk ch."""
                g0 = ch * CH
                cw = min(CH, G - g0)
                tv = rse1.tile([128, CH * OD], BF16, tag="tvs",
                               name=f"tv{it}_{ch}")
                for eng, r0, rn in chparts(cw):
                    eng.tensor_mul(
                        _pv(tv, 0, r0 * OD, [[OD, rn], [1, OD]]),
                        _pv(X_sb, 0, (g0 + r0) * OD, [[OD, rn], [1, OD]]),
                        _pv(vrep, 0, 0, [[0, rn], [1, OD]]))
                return tv

            def stageB(ch, tv):
                """tree-reduce, logit update, exp(b - max) per group."""
                g0 = ch * CH
                cw = min(CH, G - g0)
                for eng, r0, rn in chparts(cw):
                    for hw in (8, 4, 2):
                        eng.tensor_add(
                            _pv(tv, 0, r0 * OD, [[OD, rn], [O, hw], [1, O]]),
                            _pv(tv, 0, r0 * OD, [[OD, rn], [O, hw], [1, O]]),
                            _pv(tv, 0, r0 * OD + hw * O,
                                [[OD, rn], [O, hw], [1, O]]))
                    if it == 1:
                        eng.tensor_add(
                            _pv(b_sb, 0, (g0 + r0) * O, [[O, rn], [1, O]]),
                            _pv(tv, 0, r0 * OD, [[OD, rn], [1, O]]),
                            _pv(tv, 0, r0 * OD + O, [[OD, rn], [1, O]]))
                    else:
                        db = rsm.tile([128, CH * O], BF16, tag="db")
                        eng.tensor_add(
                            _pv(db, 0, r0 * O, [[O, rn], [1, O]]),
                            _pv(tv, 0, r0 * OD, [[OD, rn], [1, O]]),
                            _pv(tv, 0, r0 * OD + O, [[OD, rn], [1, O]]))
                        eng.tensor_add(
                            _pv(b_sb, 0, (g0 + r0) * O, [[1, rn * O]]),
                            _pv(b_sb, 0, (g0 + r0) * O, [[1, rn * O]]),
                            _pv(db, 0, r0 * O, [[1, rn * O]]))
                # logits are tiny (|b| < ~0.5): exp without max-shift is
                # numerically safe and keeps the max-reduce off the DVE
                eb = rse.tile([128, CH * O], BF16, tag="eb")
                s = rse.tile([128, CH], F32, tag="s")
                for gg in range(cw):
                    nc.scalar.activation(
                        _pv(eb, 0, gg * O, [[1, O]]),
                        _pv(b_sb, 0, (g0 + gg) * O, [[1, O]]),
                        AF.Exp,
                        accum_out=_pv(s, 0, gg, [[1, 1]]))
                return eb, s

            def stageC(ch, eb, s):
                """normalize c, cx = X * c, pvp accumulation."""
                g0 = ch * CH
                cw = min(CH, G - g0)
                rs = rsm.tile([128, CH], BF16, tag="rs")
                with nc.allow_low_precision(reason="softmax recip bf16"):
                    nc.vector.reciprocal(_pv(rs, 0, 0, [[1, cw]]),
                                         _pv(s, 0, 0, [[1, cw]]))
                cn = rsm.tile([128, CH * O], BF16, tag="cn")
                nc.vector.tensor_mul(
                    _pv(cn, 0, 0, [[O, cw], [1, O]]),
                    _pv(eb, 0, 0, [[O, cw], [1, O]]),
                    _pv(rs, 0, 0, [[1, cw], [0, O]]))
                cx = rse.tile([128, CH * OD], BF16, tag="cxs",
                              name=f"cx{it}_{ch}")
                for eng, r0, rn in chparts(cw):
                    eng.tensor_mul(
                        _pv(cx, 0, r0 * OD, [[OD, rn], [O, D], [1, O]]),
                        _pv(X_sb, 0, (g0 + r0) * OD, [[OD, rn], [O, D], [1, O]]),
                        _pv(cn, 0, r0 * O, [[O, rn], [0, D], [1, O]]))
                for gg in range(cw):
                    glob = g0 + gg
                    for (c0, c1) in ((0, 512), (512, 1024), (1024, 1536),
                                     (1536, OD)):
                        nc.tensor.matmul(pvp[:, c0:c1], smat_sb[:],
                                         cx[:, gg * OD + c0:gg * OD + c1],
                                         start=(glob == 0),
                                         stop=(glob == G - 1),
                                         skip_group_check=True)

            # software pipeline: tv(k+1) issues while ACT runs exps(k)
            tv = stageA(0)
            pend = None
            for ch in range(nch):
                eb, s = stageB(ch, tv)
                if ch + 1 < nch:
                    tv = stageA(ch + 1)
                stageC(ch, eb, s)
            if it == 2:
                rse1_cm.__exit__(None, None, None)
                rse_cm.__exit__(None, None, None)
                rx_cm.__exit__(None, None, None)
            v_iter(it, pvp, 1.0)
            if it == 1:
                vrep_fill()

        v_od = rt.tile([16, OD], BF16, tag="vod")
        nc.vector.tensor_copy(
            out=_pv(v_od, 0, 0, [[D, O], [1, D]]),
            in_=_pv(v_bf, 0, 0, [[1, O], [O, D]]))
        nc.sync.dma_start(out=v2d.rearrange("(p f) -> p f", f=OD),
                          in_=v_od[:])

    # =========================================================
    # Phase C: caps conv + FC head
    # =========================================================
    with (
        tc.tile_pool(name="fcw", bufs=1) as fcw,
        tc.tile_pool(name="fcs", bufs=8) as fcs,
        tc.tile_pool(name="fcb", bufs=1) as fcb,
        tc.tile_pool(name="fca", bufs=1) as fca,
    ):
        caps3 = fca.tile([3, B * OD], BF16, tag="caps3")
        for kh in range(3):
            ln = B * OD - kh * D
            nc.sync.dma_start(
                out=caps3[kh:kh + 1, 0:ln],
                in_=v2d[kh * D:kh * D + ln].rearrange("(one f) -> one f", one=1))
        w3t_sb = fcw.tile([3, 768], BF16, tag="w3t")
        nc.sync.dma_start(out=w3t_sb[:], in_=T["w3T"].ap())
        b3_sb = fcw.tile([128, 2], F32, tag="b3s")
        nc.sync.dma_start(out=b3_sb[:].rearrange("c (m one) -> c m one", one=1),
                          in_=T["b3c"].ap().rearrange("m c one -> c m one"))
        fb1_sb = fcw.tile([16, 512], F32, tag="fb1")
        nc.sync.dma_start(out=fb1_sb[:], in_=T["fb1r"].ap())
        fb2_sb = fcw.tile([128, 32], F32, tag="fb2")
        nc.sync.dma_start(out=fb2_sb[:], in_=T["fb2T"].ap())
        fb3_sb = fcw.tile([16, 102], F32, tag="fb3")
        nc.sync.dma_start(out=fb3_sb[:], in_=T["fb3r"].ap())

        with tc.tile_pool(name="fp1", bufs=2, space="PSUM") as fp1:
            h3_sb = fca.tile([128, 2 * B * 350], BF16, tag="h3")
            for mch in range(2):
                for b in range(B):
                    ps = fp1.tile([128, 512], F32, tag="ps3")
                    for kw in range(3):
                        rhs = _pv(caps3, 0, b * OD + kw,
                                  [["P", 3], [32, 50], [2, 7]])
                        nc.tensor.matmul(
                            ps[:, :350],
                            w3t_sb[:, (kw * 2 + mch) * 128:
                                   (kw * 2 + mch + 1) * 128],
                            rhs, start=(kw == 0), stop=(kw == 2))
                    nc.scalar.activation(
                        h3_sb[:, mch * B * 350 + b * 350:
                              mch * B * 350 + (b + 1) * 350],
                        ps[:, :350], AF.Relu, bias=b3_sb[:, mch:mch + 1])
            p3_sb = fca.tile([128, 2 * B * 72], BF16, tag="p3")
            for mch in range(2):
                eng = nc.vector
                def h3v(ky, kx):
                    return _pv(h3_sb, 0, mch * B * 350 + ky * 7 + kx,
                               [[350, B], [14, 24], [2, 3]])
                dst = _pv(p3_sb, 0, mch * B * 72, [[1, B], [48, 24], [16, 3]])
                eng.tensor_max(dst, h3v(0, 0), h3v(0, 1))
                for t in range(2, 9):
                    ky, kx = divmod(t, 3)
                    eng.tensor_max(dst, dst, h3v(ky, kx))
            # ---- FC1 (output-sharded, 512 cols); lhsT chunks are
            # p3_sb slices directly: chunk kt=(mch,pos) -> [128 ch, 16 b]
            psf = fp1.tile([16, 512], F32, tag="psf")
            for blk in range(18):
                fwt = fcs.tile([128, 8 * 512], BF16, tag="fwt")
                nc.sync.dma_start(
                    out=fwt[:].rearrange("p (t f) -> p t f", f=512),
                    in_=T["fw1T"][blk, :, :].rearrange("p (t f) -> p t f",
                                                       f=512))
                for sub in range(8):
                    kt = blk * 8 + sub
                    nc.tensor.matmul(psf[:],
                                     p3_sb[:, kt * B:(kt + 1) * B],
                                     fwt[:, sub * 512:(sub + 1) * 512],
                                     start=(kt == 0), stop=(kt == 143))
            f1bf = fca.tile([16, 512], BF16, tag="f1bf")
            r1f = fca.tile([16, 512], F32, tag="r1f")
            nc.vector.tensor_add(r1f[:], psf[:], fb1_sb[:])
            nc.scalar.activation(f1bf[:], r1f[:], AF.Relu)
            nc.sync.dma_start(out=_dap(f1T, 0, [[1, 16], [16, 512]]),
                              in_=f1bf[:])

        # ---- FC2 (input-sharded) + AllReduce ----
        f2l = fca.tile([128, 4 * B], BF16, tag="f2l")
        nc.sync.dma_start(
            out=f2l[:].rearrange("p (c b) -> p c b", b=B),
            in_=_dap(f1T, 0, [[16, 128], [2048, 4], [1, 16]]))
        fw2_sb = fcb.tile([128, 4 * 4096], BF16, tag="fw2")
        for cc in range(4):
            nc.sync.dma_start(out=fw2_sb[:, cc * 4096:(cc + 1) * 4096],
                              in_=T["fw2T"][cc, :, :])
        with tc.tile_pool(name="fp2", bufs=1, space="PSUM") as fp2:
            z2p = fp2.tile([16, 4096], F32, tag="z2p")
            for cc in range(4):
                lhs = f2l[:, cc * B:(cc + 1) * B]
                for j in range(8):
                    nc.tensor.matmul(
                        z2p[:, j * 512:(j + 1) * 512], lhs,
                        fw2_sb[:, cc * 4096 + j * 512:cc * 4096 + (j + 1) * 512],
                        start=(cc == 0), stop=(cc == 3))
            z2s = fca.tile([16, 4096], BF16, tag="z2s")
            nc.scalar.copy(z2s[:, 0:2048], z2p[:, 0:2048])
            nc.vector.tensor_copy(out=z2s[:, 2048:4096], in_=z2p[:, 2048:4096])
            # store transposed: z2in flat[(u, b)] = z2s[b, u]
            nc.sync.dma_start(out=_dap(z2in, 0, [[1, 16], [16, 4096]]),
                              in_=z2s[:])
        nc.gpsimd.collective_compute("AllReduce", OP.add, replica_groups=RG,
                                     ins=[z2in.opt()], outs=[z2out.opt()])
        # reload: z2T[p, cc, b] = z2out_flat[(cc*128+p)*16 + b]
        z2T = fca.tile([128, 32 * B], BF16, tag="z2T")
        nc.sync.dma_start(
            out=z2T[:].rearrange("p (c b) -> p c b", b=B),
            in_=_dap(z2out, 0, [[16, 128], [2048, 32], [1, 16]]))
        f3l = fca.tile([128, 32 * B], BF16, tag="f3l")
        nc.vector.tensor_add(
            z2T[:].rearrange("p (c b) -> p c b", b=B),
            z2T[:].rearrange("p (c b) -> p c b", b=B),
            _pv(fb2_sb, 0, 0, [[1, 32], [0, B]]))
        nc.scalar.activation(f3l[:], z2T[:], AF.Relu)

        # ---- FC3 (full, every core) ----
        fw3_sb = fcb.tile([128, 32 * 102], BF16, tag="fw3")
        nc.sync.dma_start(
            out=fw3_sb[:].rearrange("p (t f) -> p t f", f=102),
            in_=T["fw3T"].ap().rearrange("t p f -> p t f"))
        with tc.tile_pool(name="fp3", bufs=1, space="PSUM") as fp3:
            ps3f = fp3.tile([16, 512], F32, tag="ps3f")
            for cc in range(32):
                nc.tensor.matmul(ps3f[:, :102], f3l[:, cc * B:(cc + 1) * B],
                                 fw3_sb[:, cc * 102:(cc + 1) * 102],
                                 start=(cc == 0), stop=(cc == 31))
            res3 = fca.tile([16, 102], F32, tag="res3")
            nc.vector.tensor_add(res3[:], ps3f[:, :102], fb3_sb[:])
            nc.sync.dma_start(out=out_t[:, :], in_=res3[:])


def _prep_inputs(inputs):
    import ml_dtypes
    bf = ml_dtypes.bfloat16
    x = np.ascontiguousarray(inputs["x"], dtype=np.float32)
    w1, b1 = inputs["w1"], inputs["b1"]
    w2, b2 = inputs["w2"], inputs["b2"]
    wp, bp = inputs["wp"], inputs["bp"]
    Wcap = inputs["Wcap"]
    w3, b3 = inputs["w3"], inputs["b3"]
    fw1, fb1 = inputs["fw1"], inputs["fb1"]
    fw2, fb2 = inputs["fw2"], inputs["fb2"]
    fw3, fb3 = inputs["fw3"], inputs["fb3"]

    s = x.strides
    xw = as_strided(x, shape=(B, 3, 11, 11, 51, 51),
                    strides=(s[0], s[1], s[2], s[3], 4 * s[2], 4 * s[3]))
    xcols = np.ascontiguousarray(xw, dtype=bf).reshape(B, 363, 2601)

    w1T = np.ascontiguousarray(np.asarray(w1).reshape(96, 363).T, dtype=bf)
    w2T = np.ascontiguousarray(np.asarray(w2).transpose(2, 3, 1, 0),
                               dtype=bf).reshape(25, 96, 256)
    wpT = np.ascontiguousarray(np.asarray(wp).transpose(2, 3, 1, 0),
                               dtype=bf).reshape(16, 2, 128, 256)
    w3T = np.ascontiguousarray(
        np.asarray(w3).reshape(256, 9).T.reshape(3, 3, 256),
        dtype=bf).reshape(3, 768)

    Wp = np.zeros((O, IPAD, D, 8), np.float32)
    Wp[:, :ITOT] = np.asarray(Wcap)
    # d-major od columns: col = d*O + o
    wrg_all = np.ascontiguousarray(
        Wp.reshape(O, NCORES, G, 8, D, 8).transpose(1, 2, 3, 5, 4, 0),
        dtype=bf).reshape(NCORES, G, 64, OD)

    fw1 = np.asarray(fw1)
    fw2 = np.asarray(fw2)
    fw3 = np.asarray(fw3)
    # chunk kt = (mch, pos): lhsT = p3_sb[:, kt*16:(kt+1)*16] whose
    # partition p maps to f-index (mch*128 + p)*72 + pos.
    # fw1T[r][blk, p, sub*512+f] with kt = blk*8+sub = mch*72+pos
    fw1T_all = np.ascontiguousarray(
        fw1.reshape(NCORES, 512, 18432).transpose(0, 2, 1)
        .reshape(NCORES, 2, 128, 72, 512).transpose(0, 1, 3, 2, 4)
        .reshape(NCORES, 18, 8, 128, 512).transpose(0, 1, 3, 2, 4),
        dtype=bf).reshape(NCORES, 18, 128, 8 * 512)
    # fw2 input-shard: [r] -> fw2[:, 512r:512(r+1)].T -> [4, 128, 4096]
    fw2in_all = np.ascontiguousarray(
        fw2.T.reshape(NCORES, 512, 4096), dtype=bf
    ).reshape(NCORES, 4, 128, 4096)
    fw3T = np.ascontiguousarray(fw3.T.reshape(32, 128, 102), dtype=bf)
    fb2T = np.ascontiguousarray(
        np.asarray(fb2, np.float32).reshape(32, 128).T)

    shared = dict(
        w1T=w1T, b1c=np.asarray(b1, np.float32).reshape(96, 1),
        w2T=w2T, b2c=np.asarray(b2, np.float32).reshape(2, 128, 1),
        wpT=wpT, bpc=np.asarray(bp, np.float32).reshape(2, 128, 1),
        w3T=w3T, b3c=np.asarray(b3, np.float32).reshape(2, 128, 1),
        smat=np.ascontiguousarray(
            np.tile(np.eye(16, dtype=np.float32), (8, 1)), dtype=bf),
        fw3T=fw3T, fb2T=fb2T,
        fb3r=np.ascontiguousarray(
            np.tile(np.asarray(fb3, np.float32).reshape(1, 102), (16, 1))))
    in_maps = []
    for r in range(NCORES):
        m = dict(shared)
        m["xcols"] = np.ascontiguousarray(xcols[2 * r:2 * r + 2])
        m["wrg"] = np.ascontiguousarray(wrg_all[r])
        m["fw1T"] = np.ascontiguousarray(fw1T_all[r])
        m["fw2T"] = np.ascontiguousarray(fw2in_all[r])
        m["fb1r"] = np.ascontiguousarray(
            np.tile(np.asarray(fb1, np.float32)[512 * r:512 * (r + 1)]
                    .reshape(1, 512), (16, 1)))
        in_maps.append(m)
    return in_maps


def kernel(**inputs):
    if "nc" not in _CACHE:
        _CACHE["nc"] = build_program()
    in_maps = _prep_inputs(inputs)
    last_err = None
    for attempt in range(3):
        try:
            res = run_bass_kernel_spmd(_CACHE["nc"], in_maps,
                                       list(range(NCORES)))
            _CACHE["last_exec_ns"] = res.exec_time_ns
            return np.asarray(res.results[0]["out"], dtype=np.float32)
        except Exception as err:  # transient device-unrecoverable states
            last_err = err
            import time as _time
            _time.sleep(20 * (attempt + 1))
    raise last_err


# revision 5
# speedup vs baseline: 1.9155x; 1.6848x over previous
"""CapsAlexNet (FLOWER102) forward pass on 8 Trainium2 NeuronCores — v2.

Sharding (same global structure as v1, heavily bf16 + restructured):
  - conv stack: data-parallel over batch (2 images/core); conv1 via host
    im2col; all matmuls bf16 (fp32 PSUM accumulate).
  - capsule routing: capsule dim sharded 8 ways (AllToAll from batch-shard
    to i-shard). x_hat (X) computed ONCE in bf16 and kept resident in SBUF
    (17.5MB); the two logit/softmax passes run whole-X DVE ops in chunks of
    CH groups. AllReduce of [16,1632] bf16 per routing iteration (3 total).
  - caps conv computed fully on every core (v is global after AllReduce).
  - FC head: FC1 output-sharded (512 cols/core, bf16 weights streamed),
    FC2 input-sharded with a single AllReduce of the pre-activation,
    FC3 computed fully on every core.
  Collectives: AllToAll + 3x AllReduce + 1x AllReduce = 5.
"""

import numpy as np
from numpy.lib.stride_tricks import as_strided

import concourse.bass as bass
import concourse.mybir as mybir
import concourse.tile as tile
from concourse import bacc
from concourse.ap import AP
from concourse.bass_utils import run_bass_kernel_spmd

F32 = mybir.dt.float32
BF16 = mybir.dt.bfloat16
AX = mybir.AxisListType
OP = mybir.AluOpType
AF = mybir.ActivationFunctionType

NCORES = 8
B = 16
BC = 2           # images per core
O = 102
D = 16
OD = O * D       # 1632
ITOT = 2592
IPAD = 2688
ILOC = IPAD // NCORES   # 336
G = ILOC // 8           # 42 groups of 8 caps
CH = 4                  # groups per DVE chunk in routing passes
RG = [list(range(NCORES))]

_CACHE = {}


def _dap(a, offset, dims):
    """Manual AP into a dram-pool tile (which is itself an AP)."""
    return AP(tensor=a.tensor, offset=a.offset + offset,
              ap=[list(d) for d in dims])


def _pv(t, part0, free0, dims):
    """AP into SBUF tile t at (partition part0, free offset free0).

    dims: list of [step, count] free dims; prepend ("P", n) to set the
    partition count (default: full tile partitions).
    """
    base = t[:]
    fs = base.ap[0][0]          # partition stride == free size
    npart = dims[0][1] if dims[0][0] == "P" else base.ap[0][1]
    rest = dims[1:] if dims[0][0] == "P" else dims
    return AP(tensor=base.tensor, offset=base.offset + part0 * fs + free0,
              ap=[[fs, npart]] + [list(d) for d in rest])


def build_program():
    nc = bacc.Bacc("TRN2", target_bir_lowering=False, debug=False,
                   num_devices=NCORES)

    def din(name, shape, dt=F32):
        return nc.declare_dram_parameter(name, list(shape), dt, isOutput=False)

    T = dict(
        xcols=din("xcols", [BC, 363, 2601], BF16),
        w1T=din("w1T", [363, 96], BF16), b1c=din("b1c", [96, 1]),
        w2T=din("w2T", [25, 96, 256], BF16), b2c=din("b2c", [2, 128, 1]),
        wpT=din("wpT", [16, 2, 128, 256], BF16), bpc=din("bpc", [2, 128, 1]),
        wrg=din("wrg", [G, 64, OD], BF16),
        smat=din("smat", [128, 16], BF16),
        w3T=din("w3T", [3, 768], BF16), b3c=din("b3c", [2, 128, 1]),
        fw1T=din("fw1T", [18, 128, 8 * 512], BF16),
        fb1r=din("fb1r", [16, 512]),
        fw2T=din("fw2T", [4, 128, 4096], BF16),
        fb2T=din("fb2T", [128, 32]),
        fw3T=din("fw3T", [32, 128, 102], BF16),
        fb3r=din("fb3r", [16, 102]),
    )
    T["out_t"] = nc.declare_dram_parameter("out", [B, O], F32, isOutput=True)

    with tile.TileContext(nc) as tc:
        with tc.tile_pool(name="dram", bufs=1, space="DRAM") as dram:
            _build_body(nc, tc, dram, T)
    nc.finalize()
    return nc


def _build_body(nc, tc, dram, T):
    out_t = T["out_t"]

    # ---------------- DRAM scratch ----------------
    upc = dram.tile([BC, 20736], F32, tag="upc")
    u_loc = dram.tile([BC, IPAD * 8], BF16, tag="uloc")
    u_a2a = dram.tile([NCORES, BC, ILOC * 8], BF16, tag="ua2a")
    u_mine = dram.tile([NCORES, BC, ILOC * 8], BF16, tag="umine")
    u_mT = dram.tile([ILOC * 8, B], BF16, tag="umT")
    v_in = [dram.tile([16, OD], BF16, tag=f"vin{i}", name=f"vin{i}")
            for i in range(3)]
    v_out = [dram.tile([16, OD], BF16, tag=f"vout{i}", name=f"vout{i}")
             for i in range(3)]
    v2d = dram.tile([B * OD], BF16, tag="v2d")
    f1T = dram.tile([512, B], BF16, tag="f1T")
    z2in = dram.tile([16, 4096], BF16, tag="z2in")
    z2out = dram.tile([16, 4096], BF16, tag="z2out")

    # =========================================================
    # Phase A: conv stack (2 images, bf16)
    # =========================================================
    with (
        tc.tile_pool(name="caw", bufs=1) as cw,
        tc.tile_pool(name="cact", bufs=1) as ca,
        tc.tile_pool(name="cps", bufs=2, space="PSUM") as cps,
        tc.tile_pool(name="cps1", bufs=1, space="PSUM") as cps1,
    ):
        # conv1 inputs + weights first (everything else overlaps conv1)
        xc_sb = ca.tile([128, BC * 3 * 2601], BF16, tag="xc")
        for img in range(BC):
            for kt in range(3):
                rows = 128 if kt < 2 else 107
                c0 = (img * 3 + kt) * 2601
                nc.sync.dma_start(out=xc_sb[:rows, c0:c0 + 2601],
                                  in_=T["xcols"][img, kt * 128:kt * 128 + rows, :])
        w1t_sb = cw.tile([128, 3 * 96], BF16, tag="w1t")
        for kt in range(3):
            rows = 128 if kt < 2 else 107
            nc.sync.dma_start(out=w1t_sb[:rows, kt * 96:(kt + 1) * 96],
                              in_=T["w1T"][kt * 128:kt * 128 + rows, :])
        b1_sb = cw.tile([96, 1], F32, tag="b1s")
        nc.sync.dma_start(out=b1_sb[:], in_=T["b1c"][:, :])
        w2t_sb = cw.tile([96, 25 * 256], BF16, tag="w2t")
        nc.sync.dma_start(out=w2t_sb[:].rearrange("p (t o) -> p t o", o=256),
                          in_=T["w2T"].ap().rearrange("t c o -> c t o"))
        wpt_sb = cw.tile([128, 32 * 256], BF16, tag="wpt")
        nc.sync.dma_start(
            out=wpt_sb[:].rearrange("p (t k o) -> p t k o", k=2, o=256),
            in_=T["wpT"].ap().rearrange("t k c o -> c t k o"))
        b2_sb = cw.tile([128, 2], F32, tag="b2s")
        nc.sync.dma_start(out=b2_sb[:].rearrange("c (m one) -> c m one", one=1),
                          in_=T["b2c"].ap().rearrange("m c one -> c m one"))
        bp_sb = cw.tile([128, 2], F32, tag="bps")
        nc.sync.dma_start(out=bp_sb[:].rearrange("c (m one) -> c m one", one=1),
                          in_=T["bpc"].ap().rearrange("m c one -> c m one"))

        # ---- conv1 + relu ----
        h1i = [ca.tile([96, 2601], BF16, tag=f"h1_{img}",
                       name=f"h1_{img}") for img in range(BC)]
        for img in range(BC):
            for (n0, n1) in ((0, 512), (512, 1024), (1024, 1536),
                             (1536, 2048), (2048, 2560), (2560, 2601)):
                ps = cps.tile([96, 512], F32, tag="ps1")
                for kt in range(3):
                    rows = 128 if kt < 2 else 107
                    c0 = (img * 3 + kt) * 2601
                    nc.tensor.matmul(ps[:, :n1 - n0],
                                     w1t_sb[:rows, kt * 96:(kt + 1) * 96],
                                     xc_sb[:rows, c0 + n0:c0 + n1],
                                     start=(kt == 0), stop=(kt == 2))
                nc.scalar.activation(h1i[img][:, n0:n1],
                                     ps[:, :n1 - n0], AF.Relu, bias=b1_sb[:, 0:1])

        # ---- maxpool1 -> padded conv2 input ----
        p1pi = [ca.tile([96, 841], BF16, tag=f"p1p_{img}",
                        name=f"p1p_{img}") for img in range(BC)]
        for img in range(BC):
            nc.vector.memset(p1pi[img][:], 0.0)
            def h1v(ky, kx):
                return _pv(h1i[img], 0, ky * 51 + kx, [[102, 25], [2, 25]])
            dst = _pv(p1pi[img], 0, 2 * 29 + 2, [[29, 25], [1, 25]])
            nc.vector.tensor_max(dst, h1v(0, 0), h1v(0, 1))
            for t in range(2, 9):
                ky, kx = divmod(t, 3)
                nc.vector.tensor_max(dst, dst, h1v(ky, kx))

        # ---- conv2 + relu ----
        h2i = {}
        for mch in range(2):
            for img in range(BC):
                h2i[(mch, img)] = ca.tile([128, 625], BF16,
                                          tag=f"h2_{mch}_{img}",
                                          name=f"h2_{mch}_{img}")
        for mch in range(2):
            ps2 = {}
            for img in range(BC):
                for nch in range(2):
                    ps2[(img, nch)] = cps1.tile(
                        [128, 512], F32, tag=f"ps2_{img}_{nch}",
                        name=f"ps2_{mch}_{img}_{nch}")
            for tap in range(25):
                ky, kx = divmod(tap, 5)
                lhs = w2t_sb[:, tap * 256 + mch * 128:tap * 256 + (mch + 1) * 128]
                for img in range(BC):
                    for nch, (oy0, nyy) in enumerate(((0, 13), (13, 12))):
                        rhs = _pv(p1pi[img], 0, (oy0 + ky) * 29 + kx,
                                  [[29, nyy], [1, 25]])
                        nc.tensor.matmul(ps2[(img, nch)][:, :nyy * 25], lhs,
                                         rhs, start=(tap == 0), stop=(tap == 24))
            for img in range(BC):
                for nch, (oy0, nyy) in enumerate(((0, 13), (13, 12))):
                    nc.scalar.activation(
                        h2i[(mch, img)][:, oy0 * 25:(oy0 + nyy) * 25],
                        ps2[(img, nch)][:, :nyy * 25], AF.Relu,
                        bias=b2_sb[:, mch:mch + 1])

        # ---- maxpool2 ----
        p2i = {}
        for mch in range(2):
            for img in range(BC):
                p2i[(mch, img)] = ca.tile([128, 144], BF16,
                                          tag=f"p2_{mch}_{img}",
                                          name=f"p2_{mch}_{img}")
        for mch in range(2):
            for img in range(BC):
                def h2v(ky, kx):
                    return _pv(h2i[(mch, img)], 0, ky * 25 + kx,
                               [[50, 12], [2, 12]])
                d3 = p2i[(mch, img)][:].rearrange("p (a b) -> p a b", b=12)
                nc.vector.tensor_max(d3, h2v(0, 0), h2v(0, 1))
                for t in range(2, 9):
                    ky, kx = divmod(t, 3)
                    nc.vector.tensor_max(d3, d3, h2v(ky, kx))

        # ---- primarycaps conv (no relu) ----
        pc_sb = ca.tile([128, 2 * BC * 81], F32, tag="pc")
        for mch in range(2):
            psP = {img: cps1.tile([128, 81], F32, tag=f"psP_{img}",
                                  name=f"psP_{mch}_{img}")
                   for img in range(BC)}
            for tap in range(16):
                ky, kx = divmod(tap, 4)
                for kc in range(2):
                    lhs = wpt_sb[:, (tap * 2 + kc) * 256 + mch * 128:
                                 (tap * 2 + kc) * 256 + (mch + 1) * 128]
                    for img in range(BC):
                        rhs = _pv(p2i[(kc, img)], 0, ky * 12 + kx,
                                  [[12, 9], [1, 9]])
                        nc.tensor.matmul(psP[img][:], lhs, rhs,
                                         start=(tap == 0 and kc == 0),
                                         stop=(tap == 15 and kc == 1))
            for img in range(BC):
                nc.vector.tensor_scalar_add(
                    pc_sb[:, (mch * BC + img) * 81:(mch * BC + img + 1) * 81],
                    psP[img][:], bp_sb[:, mch:mch + 1])

        for mch in range(2):
            for img in range(BC):
                nc.sync.dma_start(
                    out=upc[img, mch * 128 * 81:(mch + 1) * 128 * 81]
                    .rearrange("(p f) -> p f", f=81),
                    in_=pc_sb[:, (mch * BC + img) * 81:(mch * BC + img + 1) * 81])

        # ---- squash -> u_loc (bf16) ----
        u_sb = ca.tile([128, BC * 21 * 8], F32, tag="usb")
        nc.vector.memset(u_sb[:], 0.0)
        for img in range(BC):
            nc.sync.dma_start(
                out=u_sb[:, img * 168:img * 168 + 160]
                .rearrange("p (c k) -> p c k", k=8),
                in_=_dap(upc, img * 20736, [[8, 128], [1024, 20], [1, 8]]))
            nc.sync.dma_start(
                out=u_sb[:32, img * 168 + 160:img * 168 + 168],
                in_=_dap(upc, img * 20736 + 20 * 1024, [[8, 32], [1, 8]]))
        n2 = ca.tile([128, BC * 21], F32, tag="sqn2")
        t1 = ca.tile([128, BC * 21], F32, tag="sqt1")
        r1 = ca.tile([128, BC * 21], F32, tag="sqr1")
        sq = ca.tile([128, BC * 168], F32, tag="sqsq")
        nc.scalar.activation(sq[:], u_sb[:], AF.Square)
        nc.vector.tensor_reduce(n2[:], sq[:].rearrange("p (c k) -> p c k", k=8),
                                AX.X, OP.add)
        nc.scalar.add(t1[:], n2[:], 1.0)
        nc.vector.reciprocal(t1[:], t1[:])
        nc.vector.tensor_scalar(t1[:], t1[:], -1.0, 1.0, OP.mult, OP.add)
        nc.vector.tensor_scalar_add(r1[:], n2[:], 1e-8)
        nc.scalar.activation(r1[:], r1[:], AF.Sqrt)
        nc.vector.reciprocal(r1[:], r1[:])
        nc.vector.tensor_mul(t1[:], t1[:], r1[:])
        u_bf = ca.tile([128, BC * 168], BF16, tag="ubf")
        nc.vector.tensor_mul(
            u_bf[:].rearrange("p (c k) -> p c k", k=8),
            u_sb[:].rearrange("p (c k) -> p c k", k=8),
            t1[:].rearrange("p (c one) -> p c one", one=1)
            .broadcast_to((128, BC * 21, 8)))
        for img in range(BC):
            nc.sync.dma_start(
                out=_dap(u_loc, img * 21504, [[8, 128], [1024, 21], [1, 8]]),
                in_=u_bf[:, img * 168:(img + 1) * 168]
                .rearrange("p (c k) -> p c k", k=8))

    # batch-shard -> i-shard via AllToAll (bf16 payload)
    nc.sync.dma_start(
        out=_dap(u_a2a, 0, [[5376, NCORES], [2688, BC], [1, 2688]]),
        in_=_dap(u_loc, 0, [[2688, NCORES], [21504, BC], [1, 2688]]))
    nc.gpsimd.collective_compute("AllToAll", OP.bypass, replica_groups=RG,
                                 ins=[u_a2a.opt()], outs=[u_mine.opt()])
    # u_mine as flat [16, 2688] bf16: b-major (core j's 2 images in order)

    # =========================================================
    # Phase B: routing (X resident bf16, whole-X DVE chunks)
    # =========================================================
    with (
        tc.tile_pool(name="rt", bufs=1) as rt,
        tc.tile_pool(name="rsm", bufs=1) as rsm,
        tc.tile_pool(name="rpv", bufs=1, space="PSUM") as rpv,
    ):
        rx_cm = tc.tile_pool(name="rx", bufs=1)
        rx = rx_cm.__enter__()
        X_sb = rx.tile([128, G * OD], BF16, tag="X")
        smat_sb = rt.tile([128, 16], BF16, tag="smt")
        nc.sync.dma_start(out=smat_sb[:], in_=T["smat"].ap())
        vrep = rt.tile([128, OD], BF16, tag="vrep")
        v_bf = rt.tile([16, OD], BF16, tag="vbf")
        vsum = rt.tile([16, OD], BF16, tag="vsum")

        def squash16(src, dst):
            """dst(bf16) = squash(src) over d; src [16, OD]."""
            qn2 = rsm.tile([16, O], F32, tag="q16a")
            qt = rsm.tile([16, O], F32, tag="q16b")
            qr = rsm.tile([16, O], F32, tag="q16c")
            qs = rsm.tile([16, OD], BF16, tag="q16d")
            nc.scalar.activation(qs[:], src, AF.Square)
            with nc.allow_low_precision(reason="squash norm accum"):
                nc.vector.tensor_reduce(
                    qn2[:], _pv(qs, 0, 0, [[1, O], [O, D]]), AX.X, OP.add)
            nc.scalar.add(qt[:], qn2[:], 1.0)
            nc.vector.reciprocal(qt[:], qt[:])
            nc.vector.tensor_scalar(qt[:], qt[:], -1.0, 1.0, OP.mult, OP.add)
            nc.vector.tensor_scalar_add(qr[:], qn2[:], 1e-8)
            nc.scalar.activation(qr[:], qr[:], AF.Sqrt)
            nc.vector.reciprocal(qr[:], qr[:])
            nc.vector.tensor_mul(qt[:], qt[:], qr[:])
            # d-major: dst[(d,o)] = src[(d,o)] * qt[o]
            nc.vector.tensor_mul(
                AP(tensor=dst.tensor, offset=dst.offset,
                   ap=[list(dst.ap[0]), [O, D], [1, O]]),
                AP(tensor=src.tensor, offset=src.offset,
                   ap=[list(src.ap[0]), [O, D], [1, O]]),
                _pv(qt, 0, 0, [[0, D], [1, O]]))

        def vrep_fill():
            for j in range(8):
                nc.sync.dma_start(out=vrep[j * 16:(j + 1) * 16, :],
                                  in_=v_bf[:])

        def v_iter(it, pvp, scale):
            """pvp psum [16,2048] -> AllReduce(bf16) -> squash -> v_bf."""
            vps = rsm.tile([16, OD], BF16, tag="vps")
            if scale != 1.0:
                nc.scalar.mul(vps[:], pvp[:, 0:OD], scale)
            else:
                nc.scalar.copy(vps[:], pvp[:, 0:OD])
            nc.sync.dma_start(out=v_in[it], in_=vps[:])
            nc.gpsimd.collective_compute(
                "AllReduce", OP.add, replica_groups=RG,
                ins=[v_in[it].opt()], outs=[v_out[it].opt()])
            nc.sync.dma_start(out=vsum[:], in_=v_out[it])
            squash16(vsum[:], v_bf[:])

        # ---- pass 0: build X (bf16, resident) + uniform-c v0 ----
        pvp = rpv.tile([16, 2048], F32, tag="pvp")
        with (
            tc.tile_pool(name="rtu", bufs=1) as rtu,
            tc.tile_pool(name="rws", bufs=6) as rws,
            tc.tile_pool(name="rpx", bufs=1, space="PSUM") as rpx,
        ):
            # u_mT[cap, b] = u_mine[b, cap]  (b innermost for ubd gather)
            nc.sync.dma_start(
                out=_dap(u_mT, 0, [[16, IPAD], [1, 16]]),
                in_=_dap(u_mine, 0, [[1, IPAD], [IPAD, 16]]))
            # block-diag u: ubd[c*8+k, g*128+c*16+b] = u_mT[(8g+c)*8+k, b]
            ubd = rtu.tile([64, G * 128], BF16, tag="ubd")
            nc.vector.memset(ubd[:], 0.0)
            for c in range(8):
                nc.sync.dma_start(
                    out=_pv(ubd, c * 8, c * 16, [["P", 8], [128, G], [1, 16]]),
                    in_=_dap(u_mT, 128 * c, [[16, 8], [1024, G], [1, 16]]))
            # dense u: ud[c*8+k, g*16+b] = u_mT[(8g+c)*8+k, b]
            ud = rtu.tile([64, G * 16], BF16, tag="ud")
            for c in range(8):
                nc.sync.dma_start(
                    out=_pv(ud, c * 8, 0, [["P", 8], [16, G], [1, 16]]),
                    in_=_dap(u_mT, 128 * c, [[16, 8], [1024, G], [1, 16]]))
            CK = ((0, 512), (512, 1024), (1024, 1536), (1536, OD))
            for g in range(G):
                wt = rws.tile([64, OD], BF16, tag="wt")
                nc.sync.dma_start(out=wt[:], in_=T["wrg"][g, :, :])
                lhs = ubd[:, g * 128:(g + 1) * 128]
                Xp = [rpx.tile([128, 512], F32, tag=f"Xp{j}",
                               name=f"Xp{g}_{j}") for j in range(4)]
                for j, (c0, c1) in enumerate(CK):
                    nc.tensor.matmul(Xp[j][:, :c1 - c0], lhs, wt[:, c0:c1],
                                     start=True, stop=True)
                for j, (c0, c1) in enumerate(CK):
                    if j < 2:
                        nc.scalar.copy(X_sb[:, g * OD + c0:g * OD + c1],
                                       Xp[j][:, :c1 - c0])
                    else:
                        nc.vector.tensor_copy(
                            out=X_sb[:, g * OD + c0:g * OD + c1],
                            in_=Xp[j][:, :c1 - c0])
                for (c0, c1) in CK:
                    nc.tensor.matmul(pvp[:, c0:c1],
                                     ud[:, g * 16:(g + 1) * 16],
                                     wt[:, c0:c1],
                                     start=(g == 0), stop=(g == G - 1),
                                     skip_group_check=True)
        v_iter(0, pvp, 1.0 / O)
        vrep_fill()

        # ---- passes 1, 2 ----
        b_sb = rt.tile([128, G * O], BF16, tag="blog")
        nch = (G + CH - 1) // CH
        rse_cm = tc.tile_pool(name="rse", bufs=2)
        rse = rse_cm.__enter__()
        rse1_cm = tc.tile_pool(name="rse1", bufs=1)
        rse1 = rse1_cm.__enter__()
        for it in (1, 2):
            pvp = rpv.tile([16, 2048], F32, tag="pvp")
            def chparts(cw):
                return [(nc.vector, 0, cw)]

            def stageA(ch):
                """tv = X * vrep for chunk ch."""
                g0 = ch * CH
                cw = min(CH, G - g0)
                tv = rse1.tile([128, CH * OD], BF16, tag="tvs",
                               name=f"tv{it}_{ch}")
                for eng, r0, rn in chparts(cw):
                    eng.tensor_mul(
                        _pv(tv, 0, r0 * OD, [[OD, rn], [1, OD]]),
                        _pv(X_sb, 0, (g0 + r0) * OD, [[OD, rn], [1, OD]]),
                        _pv(vrep, 0, 0, [[0, rn], [1, OD]]))
                return tv

            def stageB(ch, tv):
                """tree-reduce, logit update, exp(b - max) per group."""
                g0 = ch * CH
                cw = min(CH, G - g0)
                for eng, r0, rn in chparts(cw):
                    for hw in (8, 4, 2):
                        eng.tensor_add(
                            _pv(tv, 0, r0 * OD, [[OD, rn], [O, hw], [1, O]]),
                            _pv(tv, 0, r0 * OD, [[OD, rn], [O, hw], [1, O]]),
                            _pv(tv, 0, r0 * OD + hw * O,
                                [[OD, rn], [O, hw], [1, O]]))
                    if it == 1:
                        eng.tensor_add(
                            _pv(b_sb, 0, (g0 + r0) * O, [[O, rn], [1, O]]),
                            _pv(tv, 0, r0 * OD, [[OD, rn], [1, O]]),
                            _pv(tv, 0, r0 * OD + O, [[OD, rn], [1, O]]))
                    else:
                        db = rsm.tile([128, CH * O], BF16, tag="db")
                        eng.tensor_add(
                            _pv(db, 0, r0 * O, [[O, rn], [1, O]]),
                            _pv(tv, 0, r0 * OD, [[OD, rn], [1, O]]),
                            _pv(tv, 0, r0 * OD + O, [[OD, rn], [1, O]]))
                        eng.tensor_add(
                            _pv(b_sb, 0, (g0 + r0) * O, [[1, rn * O]]),
                            _pv(b_sb, 0, (g0 + r0) * O, [[1, rn * O]]),
                            _pv(db, 0, r0 * O, [[1, rn * O]]))
                # logits are tiny (|b| < ~0.5): exp without max-shift is
                # numerically safe and keeps the max-reduce off the DVE
                eb = rse.tile([128, CH * O], BF16, tag="eb")
                s = rse.tile([128, CH], F32, tag="s")
                for gg in range(cw):
                    nc.scalar.activation(
                        _pv(eb, 0, gg * O, [[1, O]]),
                        _pv(b_sb, 0, (g0 + gg) * O, [[1, O]]),
                        AF.Exp,
                        accum_out=_pv(s, 0, gg, [[1, 1]]))
                return eb, s

            def stageC(ch, eb, s):
                """normalize c, cx = X * c, pvp accumulation."""
                g0 = ch * CH
                cw = min(CH, G - g0)
                rs = rsm.tile([128, CH], F32, tag="rs")
                nc.vector.reciprocal(_pv(rs, 0, 0, [[1, cw]]),
                                     _pv(s, 0, 0, [[1, cw]]))
                cn = rsm.tile([128, CH * O], BF16, tag="cn")
                for gg in range(cw):
                    nc.scalar.activation(
                        _pv(cn, 0, gg * O, [[1, O]]),
                        _pv(eb, 0, gg * O, [[1, O]]),
                        AF.Copy, scale=_pv(rs, 0, gg, [[1, 1]]))
                cx = rse.tile([128, CH * OD], BF16, tag="cxs",
                              name=f"cx{it}_{ch}")
                for eng, r0, rn in chparts(cw):
                    eng.tensor_mul(
                        _pv(cx, 0, r0 * OD, [[OD, rn], [O, D], [1, O]]),
                        _pv(X_sb, 0, (g0 + r0) * OD, [[OD, rn], [O, D], [1, O]]),
                        _pv(cn, 0, r0 * O, [[O, rn], [0, D], [1, O]]))
                for gg in range(cw):
                    glob = g0 + gg
                    for (c0, c1) in ((0, 512), (512, 1024), (1024, 1536),
                                     (1536, OD)):
                        nc.tensor.matmul(pvp[:, c0:c1], smat_sb[:],
                                         cx[:, gg * OD + c0:gg * OD + c1],
                                         start=(glob == 0),
                                         stop=(glob == G - 1),
                                         skip_group_check=True)

            # software pipeline: tv(k+1) issues while ACT runs exps(k)
            tv = stageA(0)
            pend = None
            for ch in range(nch):
                eb, s = stageB(ch, tv)
                if ch + 1 < nch:
                    tv = stageA(ch + 1)
                stageC(ch, eb, s)
            if it == 2:
                rse1_cm.__exit__(None, None, None)
                rse_cm.__exit__(None, None, None)
                rx_cm.__exit__(None, None, None)
            v_iter(it, pvp, 1.0)
            if it == 1:
                vrep_fill()

        v_od = rt.tile([16, OD], BF16, tag="vod")
        nc.vector.tensor_copy(
            out=_pv(v_od, 0, 0, [[D, O], [1, D]]),
            in_=_pv(v_bf, 0, 0, [[1, O], [O, D]]))
        nc.sync.dma_start(out=v2d.rearrange("(p f) -> p f", f=OD),
                          in_=v_od[:])

    # =========================================================
    # Phase C: caps conv + FC head
    # =========================================================
    with (
        tc.tile_pool(name="fcw", bufs=1) as fcw,
        tc.tile_pool(name="fcs", bufs=8) as fcs,
        tc.tile_pool(name="fcb", bufs=1) as fcb,
        tc.tile_pool(name="fca", bufs=1) as fca,
    ):
        caps3 = fca.tile([3, B * OD], BF16, tag="caps3")
        for kh in range(3):
            ln = B * OD - kh * D
            nc.sync.dma_start(
                out=caps3[kh:kh + 1, 0:ln],
                in_=v2d[kh * D:kh * D + ln].rearrange("(one f) -> one f", one=1))
        w3t_sb = fcw.tile([3, 768], BF16, tag="w3t")
        nc.sync.dma_start(out=w3t_sb[:], in_=T["w3T"].ap())
        b3_sb = fcw.tile([128, 2], F32, tag="b3s")
        nc.sync.dma_start(out=b3_sb[:].rearrange("c (m one) -> c m one", one=1),
                          in_=T["b3c"].ap().rearrange("m c one -> c m one"))
        fb1_sb = fcw.tile([16, 512], F32, tag="fb1")
        nc.sync.dma_start(out=fb1_sb[:], in_=T["fb1r"].ap())
        fb2_sb = fcw.tile([128, 32], F32, tag="fb2")
        nc.sync.dma_start(out=fb2_sb[:], in_=T["fb2T"].ap())
        fb3_sb = fcw.tile([16, 102], F32, tag="fb3")
        nc.sync.dma_start(out=fb3_sb[:], in_=T["fb3r"].ap())

        with tc.tile_pool(name="fp1", bufs=2, space="PSUM") as fp1:
            h3_sb = fca.tile([128, 2 * B * 350], BF16, tag="h3")
            for mch in range(2):
                for b in range(B):
                    ps = fp1.tile([128, 512], F32, tag="ps3")
                    for kw in range(3):
                        rhs = _pv(caps3, 0, b * OD + kw,
                                  [["P", 3], [32, 50], [2, 7]])
                        nc.tensor.matmul(
                            ps[:, :350],
                            w3t_sb[:, (kw * 2 + mch) * 128:
                                   (kw * 2 + mch + 1) * 128],
                            rhs, start=(kw == 0), stop=(kw == 2))
                    nc.scalar.activation(
                        h3_sb[:, mch * B * 350 + b * 350:
                              mch * B * 350 + (b + 1) * 350],
                        ps[:, :350], AF.Relu, bias=b3_sb[:, mch:mch + 1])
            p3_sb = fca.tile([128, 2 * B * 72], BF16, tag="p3")
            for mch in range(2):
                eng = nc.vector
                def h3v(ky, kx):
                    return _pv(h3_sb, 0, mch * B * 350 + ky * 7 + kx,
                               [[350, B], [14, 24], [2, 3]])
                dst = _pv(p3_sb, 0, mch * B * 72, [[1, B], [48, 24], [16, 3]])
                eng.tensor_max(dst, h3v(0, 0), h3v(0, 1))
                for t in range(2, 9):
                    ky, kx = divmod(t, 3)
                    eng.tensor_max(dst, dst, h3v(ky, kx))
            # ---- FC1 (output-sharded, 512 cols); lhsT chunks are
            # p3_sb slices directly: chunk kt=(mch,pos) -> [128 ch, 16 b]
            psf = fp1.tile([16, 512], F32, tag="psf")
            for blk in range(18):
                fwt = fcs.tile([128, 8 * 512], BF16, tag="fwt")
                nc.sync.dma_start(
                    out=fwt[:].rearrange("p (t f) -> p t f", f=512),
                    in_=T["fw1T"][blk, :, :].rearrange("p (t f) -> p t f",
                                                       f=512))
                for sub in range(8):
                    kt = blk * 8 + sub
                    nc.tensor.matmul(psf[:],
                                     p3_sb[:, kt * B:(kt + 1) * B],
                                     fwt[:, sub * 512:(sub + 1) * 512],
                                     start=(kt == 0), stop=(kt == 143))
            f1bf = fca.tile([16, 512], BF16, tag="f1bf")
            r1f = fca.tile([16, 512], F32, tag="r1f")
            nc.vector.tensor_add(r1f[:], psf[:], fb1_sb[:])
            nc.scalar.activation(f1bf[:], r1f[:], AF.Relu)
            nc.sync.dma_start(out=_dap(f1T, 0, [[1, 16], [16, 512]]),
                              in_=f1bf[:])

        # ---- FC2 (input-sharded) + AllReduce ----
        f2l = fca.tile([128, 4 * B], BF16, tag="f2l")
        nc.sync.dma_start(
            out=f2l[:].rearrange("p (c b) -> p c b", b=B),
            in_=_dap(f1T, 0, [[16, 128], [2048, 4], [1, 16]]))
        fw2_sb = fcb.tile([128, 4 * 4096], BF16, tag="fw2")
        for cc in range(4):
            nc.sync.dma_start(out=fw2_sb[:, cc * 4096:(cc + 1) * 4096],
                              in_=T["fw2T"][cc, :, :])
        with tc.tile_pool(name="fp2", bufs=1, space="PSUM") as fp2:
            z2p = fp2.tile([16, 4096], F32, tag="z2p")
            for cc in range(4):
                lhs = f2l[:, cc * B:(cc + 1) * B]
                for j in range(8):
                    nc.tensor.matmul(
                        z2p[:, j * 512:(j + 1) * 512], lhs,
                        fw2_sb[:, cc * 4096 + j * 512:cc * 4096 + (j + 1) * 512],
                        start=(cc == 0), stop=(cc == 3))
            z2s = fca.tile([16, 4096], BF16, tag="z2s")
            nc.scalar.copy(z2s[:, 0:2048], z2p[:, 0:2048])
            nc.vector.tensor_copy(out=z2s[:, 2048:4096], in_=z2p[:, 2048:4096])
            # store transposed: z2in flat[(u, b)] = z2s[b, u]
            nc.sync.dma_start(out=_dap(z2in, 0, [[1, 16], [16, 4096]]),
                              in_=z2s[:])
        nc.gpsimd.collective_compute("AllReduce", OP.add, replica_groups=RG,
                                     ins=[z2in.opt()], outs=[z2out.opt()])
        # reload: z2T[p, cc, b] = z2out_flat[(cc*128+p)*16 + b]
        z2T = fca.tile([128, 32 * B], BF16, tag="z2T")
        nc.sync.dma_start(
            out=z2T[:].rearrange("p (c b) -> p c b", b=B),
            in_=_dap(z2out, 0, [[16, 128], [2048, 32], [1, 16]]))
        f3l = fca.tile([128, 32 * B], BF16, tag="f3l")
        nc.vector.tensor_add(
            z2T[:].rearrange("p (c b) -> p c b", b=B),
            z2T[:].rearrange("p (c b) -> p c b", b=B),
            _pv(fb2_sb, 0, 0, [[1, 32], [0, B]]))
        nc.scalar.activation(f3l[:], z2T[:], AF.Relu)

        # ---- FC3 (full, every core) ----
        fw3_sb = fcb.tile([128, 32 * 102], BF16, tag="fw3")
        nc.sync.dma_start(
            out=fw3_sb[:].rearrange("p (t f) -> p t f", f=102),
            in_=T["fw3T"].ap().rearrange("t p f -> p t f"))
        with tc.tile_pool(name="fp3", bufs=1, space="PSUM") as fp3:
            ps3f = fp3.tile([16, 512], F32, tag="ps3f")
            for cc in range(32):
                nc.tensor.matmul(ps3f[:, :102], f3l[:, cc * B:(cc + 1) * B],
                                 fw3_sb[:, cc * 102:(cc + 1) * 102],
                                 start=(cc == 0), stop=(cc == 31))
            res3 = fca.tile([16, 102], F32, tag="res3")
            nc.vector.tensor_add(res3[:], ps3f[:, :102], fb3_sb[:])
            nc.sync.dma_start(out=out_t[:, :], in_=res3[:])


def _prep_inputs(inputs):
    import ml_dtypes
    bf = ml_dtypes.bfloat16
    x = np.ascontiguousarray(inputs["x"], dtype=np.float32)
    w1, b1 = inputs["w1"], inputs["b1"]
    w2, b2 = inputs["w2"], inputs["b2"]
    wp, bp = inputs["wp"], inputs["bp"]
    Wcap = inputs["Wcap"]
    w3, b3 = inputs["w3"], inputs["b3"]
    fw1, fb1 = inputs["fw1"], inputs["fb1"]
    fw2, fb2 = inputs["fw2"], inputs["fb2"]
    fw3, fb3 = inputs["fw3"], inputs["fb3"]

    s = x.strides
    xw = as_strided(x, shape=(B, 3, 11, 11, 51, 51),
                    strides=(s[0], s[1], s[2], s[3], 4 * s[2], 4 * s[3]))
    xcols = np.ascontiguousarray(xw, dtype=bf).reshape(B, 363, 2601)

    w1T = np.ascontiguousarray(np.asarray(w1).reshape(96, 363).T, dtype=bf)
    w2T = np.ascontiguousarray(np.asarray(w2).transpose(2, 3, 1, 0),
                               dtype=bf).reshape(25, 96, 256)
    wpT = np.ascontiguousarray(np.asarray(wp).transpose(2, 3, 1, 0),
                               dtype=bf).reshape(16, 2, 128, 256)
    w3T = np.ascontiguousarray(
        np.asarray(w3).reshape(256, 9).T.reshape(3, 3, 256),
        dtype=bf).reshape(3, 768)

    Wp = np.zeros((O, IPAD, D, 8), np.float32)
    Wp[:, :ITOT] = np.asarray(Wcap)
    # d-major od columns: col = d*O + o
    wrg_all = np.ascontiguousarray(
        Wp.reshape(O, NCORES, G, 8, D, 8).transpose(1, 2, 3, 5, 4, 0),
        dtype=bf).reshape(NCORES, G, 64, OD)

    fw1 = np.asarray(fw1)
    fw2 = np.asarray(fw2)
    fw3 = np.asarray(fw3)
    # chunk kt = (mch, pos): lhsT = p3_sb[:, kt*16:(kt+1)*16] whose
    # partition p maps to f-index (mch*128 + p)*72 + pos.
    # fw1T[r][blk, p, sub*512+f] with kt = blk*8+sub = mch*72+pos
    fw1T_all = np.ascontiguousarray(
        fw1.reshape(NCORES, 512, 18432).transpose(0, 2, 1)
        .reshape(NCORES, 2, 128, 72, 512).transpose(0, 1, 3, 2, 4)
        .reshape(NCORES, 18, 8, 128, 512).transpose(0, 1, 3, 2, 4),
        dtype=bf).reshape(NCORES, 18, 128, 8 * 512)
    # fw2 input-shard: [r] -> fw2[:, 512r:512(r+1)].T -> [4, 128, 4096]
    fw2in_all = np.ascontiguousarray(
        fw2.T.reshape(NCORES, 512, 4096), dtype=bf
    ).reshape(NCORES, 4, 128, 4096)
    fw3T = np.ascontiguousarray(fw3.T.reshape(32, 128, 102), dtype=bf)
    fb2T = np.ascontiguousarray(
        np.asarray(fb2, np.float32).reshape(32, 128).T)

    shared = dict(
        w1T=w1T, b1c=np.asarray(b1, np.float32).reshape(96, 1),
        w2T=w2T, b2c=np.asarray(b2, np.float32).reshape(2, 128, 1),
        wpT=wpT, bpc=np.asarray(bp, np.float32).reshape(2, 128, 1),
        w3T=w3T, b3c=np.asarray(b3, np.float32).reshape(2, 128, 1),
        smat=np.ascontiguousarray(
            np.tile(np.eye(16, dtype=np.float32), (8, 1)), dtype=bf),
        fw3T=fw3T, fb2T=fb2T,
        fb3r=np.ascontiguousarray(
            np.tile(np.asarray(fb3, np.float32).reshape(1, 102), (16, 1))))
    in_maps = []
    for r in range(NCORES):
        m = dict(shared)
        m["xcols"] = np.ascontiguousarray(xcols[2 * r:2 * r + 2])
        m["wrg"] = np.ascontiguousarray(wrg_all[r])
        m["fw1T"] = np.ascontiguousarray(fw1T_all[r])
        m["fw2T"] = np.ascontiguousarray(fw2in_all[r])
        m["fb1r"] = np.ascontiguousarray(
            np.tile(np.asarray(fb1, np.float32)[512 * r:512 * (r + 1)]
                    .reshape(1, 512), (16, 1)))
        in_maps.append(m)
    return in_maps


def kernel(**inputs):
    if "nc" not in _CACHE:
        _CACHE["nc"] = build_program()
    in_maps = _prep_inputs(inputs)
    last_err = None
    for attempt in range(3):
        try:
            res = run_bass_kernel_spmd(_CACHE["nc"], in_maps,
                                       list(range(NCORES)))
            _CACHE["last_exec_ns"] = res.exec_time_ns
            return np.asarray(res.results[0]["out"], dtype=np.float32)
        except Exception as err:  # transient device-unrecoverable states
            last_err = err
            import time as _time
            _time.sleep(20 * (attempt + 1))
    raise last_err
